# revision 59
# baseline (speedup 1.0000x reference)
"""BiLSTM-CRF sequence-tagging loss on 8 Trainium2 NeuronCores.

Sharding: pure data-parallel — core c owns sequences [4c, 4c+4) and runs
BOTH LSTM directions locally (no collectives at all).  The backward
direction writes its h-stream time-reversed directly (free AP offset), so
layer-1/emission inputs are plain [h_fwd | h_bwd] concats.

Per-step recurrence structure (per direction, chains interleaved so the
two directions hide each other's latency):
  PE   : G+bias injected into PSUM via identity matmul (prefetched one
         step ahead into the alternate bank) + 36 Whh matmuls.
  ACT  : one Sigmoid over all 12 gate chunks — the g-gate uses
         tanh(x) = 2*sigmoid(2x) - 1 with the 2x folded into the weights.
  DVE  : u = (sg - 0.5) * si ; t1 = sf * c_prev ; c = 2u + t1 (fused
         scalar_tensor_tensor ops).
  ACT  : tc = tanh(c)
  DVE  : h = so * tc  (written straight into the bf16 h stream).
"""

import os
import sys

import numpy as np

for _p in ("/opt/trn_rl_repo", "/root/.axon_site/_ro/trn_rl_repo"):
    if os.path.isdir(_p) and _p not in sys.path:
        sys.path.insert(0, _p)

import ml_dtypes  # noqa: E402

import concourse.bass as bass  # noqa: E402
import concourse.bacc as bacc  # noqa: E402
import concourse.tile as tile  # noqa: E402
from concourse import mybir  # noqa: E402
from concourse.bass import IndirectOffsetOnAxis  # noqa: E402
from concourse.bass_utils import run_bass_kernel_spmd  # noqa: E402
from concourse.masks import make_identity  # noqa: E402

F32 = mybir.dt.float32
BF16 = mybir.dt.bfloat16
I32 = mybir.dt.int32
AF = mybir.ActivationFunctionType
ALU = mybir.AluOpType

# problem shapes (hardcoded per contract)
B, T, V, D, C, HD = 32, 256, 30522, 768, 14, 384
L = 2
NCORES = 8
GB = 4             # sequences per core
NT = GB * T        # tokens per core = 1024
NTILE = NT // 128  # 8
MCH = 12           # gate chunks of 128 per direction (4*HD/128)
KCH = 3            # hidden chunks per direction (HD/128)
DCH = 6            # input-dim chunks (D/128)
LN_EPS = 1e-12
RENORM = 8

DEBUG_OUTS = False


def _bf(x):
    return np.ascontiguousarray(np.asarray(x, dtype=np.float32)).astype(ml_dtypes.bfloat16)


def _f32(x):
    return np.ascontiguousarray(np.asarray(x, dtype=np.float32))


# ---------------------------------------------------------------------------
# device program
# ---------------------------------------------------------------------------

def build_program():
    nc = bacc.Bacc("TRN2", target_bir_lowering=False, debug=False, num_devices=NCORES)

    def din(name, shape, dt):
        return nc.dram_tensor(name, shape, dt, kind="ExternalInput").ap()

    ins = dict(
        ids32=din("ids32", [NT, 1], I32),
        labf=din("labf", [1, NT], F32),
        word_emb=din("word_emb", [V, D], F32),
        posty=din("posty", [T, D], F32),
        wih0T=din("wih0T", [D, 2 * 4 * HD], BF16),
        wih1T=din("wih1T", [D, 2 * 4 * HD], BF16),
        whh0T=din("whh0T", [HD, 2 * 4 * HD], BF16),
        whh1T=din("whh1T", [HD, 2 * 4 * HD], BF16),
        b01=din("b01", [128, 4 * MCH], F32),
        fcT=din("fcT", [D, C], BF16),
        cpack=din("cpack", [C, 48], F32),
    )

    loss_out = nc.dram_tensor("loss", [1, 1], F32, kind="ExternalOutput").ap()
    dbg = {}
    if DEBUG_OUTS:
        dbg["dbg_xt"] = nc.dram_tensor("dbg_xt", [128, DCH, NT], BF16, kind="ExternalOutput").ap()
        dbg["dbg_g"] = nc.dram_tensor("dbg_g", [128, 2 * MCH, NT], BF16, kind="ExternalOutput").ap()
        dbg["dbg_h0f"] = nc.dram_tensor("dbg_h0f", [128, KCH, NT], BF16, kind="ExternalOutput").ap()
        dbg["dbg_h0b"] = nc.dram_tensor("dbg_h0b", [128, KCH, NT], BF16, kind="ExternalOutput").ap()
        dbg["dbg_h1f"] = nc.dram_tensor("dbg_h1f", [128, KCH, NT], BF16, kind="ExternalOutput").ap()
        dbg["dbg_h1b"] = nc.dram_tensor("dbg_h1b", [128, KCH, NT], BF16, kind="ExternalOutput").ap()
        dbg["dbg_em"] = nc.dram_tensor("dbg_em", [C, NT], F32, kind="ExternalOutput").ap()
        dbg["dbg_sc"] = nc.dram_tensor("dbg_sc", [1, 2], F32, kind="ExternalOutput").ap()

    with tile.TileContext(nc) as tc:
        _build_body(tc, ins, loss_out, dbg)

    nc.compile()
    return nc


def _build_body(tc, ins, loss_out, dbg):
    nc = tc.nc
    from contextlib import ExitStack

    est = ExitStack()
    pers = est.enter_context(tc.tile_pool(name="pers", bufs=1))

    # scratch + absorbers: only ONE sem wait per instruction is allowed, so
    # junctions of two producers get a tiny absorber op that folds one
    # producer into the consuming engine's clock first.
    scr_dve = pers.tile([1, 4], F32, name="scr_dve")
    scr_act = pers.tile([1, 4], F32, name="scr_act")
    pabs = est.enter_context(tc.tile_pool(name="pabs", bufs=1, space="PSUM"))
    pscr = pabs.tile([1, 8], F32, name="pscr")

    scr_gp = pers.tile([1, 4], F32, name="scr_gp")

    def dve_touch(ap):
        nc.vector.tensor_copy(out=scr_dve[:, 0:1], in_=ap)

    def gp_touch(ap):
        nc.gpsimd.tensor_copy(out=scr_gp[:, 0:1], in_=ap)

    def act_touch(ap):
        nc.scalar.activation(out=scr_act[:, 0:1], in_=ap, func=AF.Copy)

    def pe_touch(ap_col):
        nc.tensor.matmul(out=pscr[:1, :1], lhsT=ap_col, rhs=ap_col, start=True, stop=True)

    b_sb = pers.tile([128, 4 * MCH], F32, name="b_sb")
    nc.sync.dma_start(out=b_sb[:], in_=ins["b01"])
    dve_touch(b_sb[0:1, 0:1])
    act_touch(b_sb[0:1, 0:1])

    fcT_sb = pers.tile([128, DCH, C], BF16, name="fcT")
    nc.sync.dma_start(out=fcT_sb[:], in_=ins["fcT"].rearrange("(k p) m -> p k m", p=128))

    cpack_sb = pers.tile([C, 48], F32, name="cpack_sb")
    nc.sync.dma_start(out=cpack_sb[:], in_=ins["cpack"])
    dve_touch(cpack_sb[0:1, 0:1])
    E_sb = cpack_sb[:, 0:C]
    transT_sb = cpack_sb[:, C : 2 * C]
    expst_sb = cpack_sb[:, 28:29]
    expen_sb = cpack_sb[:, 29:30]
    stv_sb = cpack_sb[:, 30:31]
    env_sb = cpack_sb[:, 31:32]
    iota_sb = cpack_sb[:, 32:33]
    fcb_sb = cpack_sb[:, 33:34]
    expTT_sb = cpack_sb[:, 34:48]

    ids_sb = pers.tile([128, NTILE], I32, name="ids_sb")
    nc.sync.dma_start(out=ids_sb[:], in_=ins["ids32"].rearrange("(k p) o -> p (k o)", p=128))

    ident = pers.tile([128, 128], F32, name="ident")
    make_identity(nc, ident[:])
    pe_touch(ident[:, 0:1])
    identb = pers.tile([128, 128], BF16, name="identb")
    make_identity(nc, identb[:])
    pe_touch(identb[:, 0:1])
    eps_sb = pers.tile([128, 1], F32, name="eps_sb")
    nc.vector.memset(eps_sb[:], LN_EPS)
    ones1C = pers.tile([1, C], F32, name="ones1C")
    nc.vector.memset(ones1C[:], 1.0)
    onesC1 = pers.tile([C, 1], F32, name="onesC1")
    nc.vector.memset(onesC1[:], 1.0)

    # ---- helpers ----
    def s1_embed(xT_sb, ks, posty_sb):
        with tc.tile_pool(name="s1", bufs=3) as s1, tc.tile_pool(
            name="s1ps", bufs=2, space="PSUM"
        ) as s1ps:
            for k in ks:
                emb = s1.tile([128, D], F32, tag="emb")
                nc.gpsimd.indirect_dma_start(
                    out=emb[:],
                    out_offset=None,
                    in_=ins["word_emb"],
                    in_offset=IndirectOffsetOnAxis(ap=ids_sb[:, k : k + 1], axis=0),
                )
                nc.vector.tensor_add(out=emb[:], in0=emb[:], in1=posty_sb[:, k % 2, :])
                stats = s1.tile([128, 3, 6], F32, tag="stats")
                embv = emb[:].rearrange("p (s q) -> p s q", s=3)
                for sg_ in range(3):
                    nc.vector.bn_stats(out=stats[:, sg_, :], in_=embv[:, sg_, :])
                mv = s1.tile([128, 2], F32, tag="mv")
                nc.vector.bn_aggr(out=mv[:], in_=stats[:])
                std = s1.tile([128, 1], F32, tag="std")
                nc.scalar.activation(out=std[:], in_=mv[:, 1:2], func=AF.Sqrt, bias=eps_sb[:])
                rstd = s1.tile([128, 1], F32, tag="rstd")
                nc.vector.reciprocal(out=rstd[:], in_=std[:])
                xln = s1.tile([128, D], F32, tag="xln")
                nc.vector.tensor_scalar(
                    out=xln[:],
                    in0=emb[:],
                    scalar1=mv[:, 0:1],
                    scalar2=rstd[:],
                    op0=ALU.subtract,
                    op1=ALU.mult,
                )
                kl = k - ks[0]
                for j in range(DCH):
                    tp = s1ps.tile([128, 128], F32, tag="tp")
                    nc.tensor.transpose(
                        out=tp[:], in_=xln[:, 128 * j : 128 * (j + 1)], identity=ident[:]
                    )
                    if j % 2 == 0:
                        nc.vector.tensor_copy(
                            out=xT_sb[:, j, 128 * kl : 128 * (kl + 1)], in_=tp[:]
                        )
                    else:
                        nc.scalar.activation(
                            out=xT_sb[:, j, 128 * kl : 128 * (kl + 1)], in_=tp[:], func=AF.Copy
                        )

    def g_matmul(l, G_sb, rhs_of, wih, nbs=None, tag="", ncol=512):
        """G = x @ Wih^T + bias for both dirs, written into G_sb (bf16)."""
        with tc.tile_pool(name=f"g{l}ps{tag}", bufs=4, space="PSUM") as gps:
            nc.tensor.ldweights(weights=wih[:, 0, 0:1])
            for m in range(2 * MCH):
                for nb in (range(NT // ncol) if nbs is None else nbs):
                    ps = gps.tile([128, ncol], F32, tag="gps", name="gps")
                    for kk in range(DCH):
                        nc.tensor.matmul(
                            out=ps[:],
                            lhsT=wih[:, kk, 128 * m : 128 * (m + 1)],
                            rhs=rhs_of(kk, nb),
                            start=(kk == 0),
                            stop=(kk == DCH - 1),
                        )
                    bcol = b_sb[:, l * 2 * MCH + m : l * 2 * MCH + m + 1]
                    if (m + nb) % 2 == 0:
                        nc.vector.tensor_scalar_add(
                            out=G_sb[:, m, ncol * nb : ncol * (nb + 1)],
                            in0=ps[:],
                            scalar1=bcol,
                        )
                    else:
                        nc.scalar.activation(
                            out=G_sb[:, m, ncol * nb : ncol * (nb + 1)],
                            in_=ps[:],
                            func=AF.Identity,
                            bias=bcol,
                        )

    def recurrence(l, G_sb, hf, hb, whh, bias_rep=None, stream=None):
        """Both-direction LSTM over local time, staggered chains.

        Direction d reads G chunks [12d:12d+12] at time col (fwd: t, bwd:
        T-1-t) and writes its h at the same time col -> both h streams end
        up time-ordered.

        bias_rep: [128, 24, GB] bias tile injected into PSUM with G (used
        when G_sb itself carries no bias).
        stream: (G1_sb, wih1, spool) — accumulate the NEXT layer's G into
        G1_sb column-by-column as h values appear (fills PE/DVE idle)."""
        Gv = G_sb[:].rearrange("p m (b t) -> p m b t", b=GB)
        hv = [h[:].rearrange("p c (b t) -> p c b t", b=GB) for h in (hf, hb)]
        if stream is not None:
            G1_sb, wih1s, spool, bias1s = stream
            G1v = G1_sb[:].rearrange("p m (b t) -> p m b t", b=GB)
            nc.tensor.ldweights(weights=wih1s[:, 0, 0:1])

        with tc.tile_pool(name=f"r{l}", bufs=3) as rp, tc.tile_pool(
            name=f"r{l}c", bufs=2
        ) as rcp, tc.tile_pool(
            name=f"r{l}psA", bufs=2, space="PSUM"
        ) as rpsA, tc.tile_pool(
            name=f"r{l}psB", bufs=2, space="PSUM"
        ) as rpsB:
            rps = [rpsA, rpsB]
            ps_cur = [None, None]   # PSUM tile holding this step's preacts
            ps_nxt = [None, None]
            c_prev = [None, None]
            nc.tensor.ldweights(weights=whh[:, 0, 0:1])

            stream_pending = []
            sps_done = []

            def tcol(d, t):
                return t if d == 0 else T - 1 - t

            def emit_stream_mm(d, col):
                sp = spool.tile([128, 2 * MCH, GB], F32, tag=f"sp{d}", name="sp")
                first = (col <= T // 2 - 1) if d == 0 else (col >= T // 2)
                # open the accumulation with the bias (first touch) or the
                # column's current partial sum (second touch), so the final
                # move is a plain copy on either engine.
                nc.tensor.matmul(
                    out=sp[:],
                    lhsT=identb[:],
                    rhs=(bias1s[:] if first else G1v[:, :, :, col]),
                    start=True,
                    stop=False,
                    skip_group_check=True,
                )
                for m in range(2 * MCH):
                    for kj in range(KCH):
                        nc.tensor.matmul(
                            out=sp[:, m, :],
                            lhsT=wih1s[:, kj + KCH * d, 128 * m : 128 * (m + 1)],
                            rhs=hv[d][:, kj, :, col],
                            start=False,
                            stop=(kj == KCH - 1),
                            skip_group_check=True,
                        )
                return sp

            def emit_stream_move(d, col, sp):
                # plain copy, alternated across ACT/DVE by direction so each
                # engine absorbs one move per step in its idle window.
                if d == 0:
                    nc.scalar.activation(out=G1v[:, :, :, col], in_=sp[:], func=AF.Copy)
                else:
                    nc.vector.tensor_copy(out=G1v[:, :, :, col], in_=sp[:])

            def emit_inject(d, tt, close):
                psn = rps[d].tile([128, MCH, GB], F32, tag=f"ps{d}", name=f"psd")
                nc.tensor.matmul(
                    out=psn[:],
                    lhsT=identb[:],
                    rhs=Gv[:, MCH * d : MCH * (d + 1), :, tcol(d, tt)],
                    start=True,
                    stop=(close and bias_rep is None),
                    skip_group_check=True,
                )
                if bias_rep is not None:
                    nc.tensor.matmul(
                        out=psn[:],
                        lhsT=ident[:],
                        rhs=bias_rep[:, MCH * d : MCH * (d + 1), :],
                        start=False,
                        stop=close,
                        skip_group_check=True,
                    )
                return psn

            if bias_rep is not None:
                # t=0 preacts also come via PSUM (G_sb carries no bias)
                for d in (0, 1):
                    ps_cur[d] = emit_inject(d, 0, close=True)

            for t in range(T):
                sg = [None, None]
                # --- flush stream moves whose PSUM inputs are ready; at the
                # top of the cycle both ACT and DVE are idle waiting on the
                # chain, so these are free ---
                if stream is not None:
                    for d, col, sp in sps_done:
                        emit_stream_move(d, col, sp)
                    sps_done = []
                # --- PE: Whh matmuls into the prefetched PSUM bank ---
                if t > 0:
                    for d in (0, 1):
                        ps = ps_cur[d]
                        hprev = hv[d][:, :, :, tcol(d, t - 1)]
                        for kk in range(KCH):
                            for m in range(MCH):
                                nc.tensor.matmul(
                                    out=ps[:, m, :],
                                    lhsT=whh[
                                        :, kk, 1536 * d + 128 * m : 1536 * d + 128 * (m + 1)
                                    ],
                                    rhs=hprev[:, kk, :],
                                    start=False,
                                    stop=(kk == KCH - 1 and m == MCH - 1),
                                    skip_group_check=True,
                                )
                # --- ACT: sigmoid over all 12 gate chunks (both dirs) ---
                for d in (0, 1):
                    sg[d] = rp.tile([128, MCH, GB], F32, tag=f"sg{d}", name=f"sg{d}")
                    if t == 0 and bias_rep is None:
                        src = Gv[:, MCH * d : MCH * (d + 1), :, tcol(d, 0)]
                    else:
                        src = ps_cur[d][:]
                    nc.scalar.activation(out=sg[d][:], in_=src, func=AF.Sigmoid)
                # --- PE: prefetch next step's G+bias into the other bank ---
                if t + 1 < T:
                    for d in (0, 1):
                        ps_nxt[d] = emit_inject(d, t + 1, close=False)
                # --- stream PREVIOUS step's next-layer G column (its h sem
                # is long satisfied, so this runs in PE's idle window after
                # the critical whh burst rather than delaying it) ---
                if stream is not None:
                    for d, col in stream_pending:
                        sps_done.append((d, col, emit_stream_mm(d, col)))
                    stream_pending = []
                # --- DVE: c update ---
                c_new = [None, None]
                for d in (0, 1):
                    u = rp.tile([128, KCH, GB], F32, tag=f"u{d}", name=f"ud")
                    nc.vector.scalar_tensor_tensor(
                        out=u[:],
                        in0=sg[d][:, 6:9, :],
                        scalar=0.5,
                        in1=sg[d][:, 0:3, :],
                        op0=ALU.subtract,
                        op1=ALU.mult,
                    )
                    c_new[d] = rcp.tile([128, KCH, GB], F32, tag=f"c{d}", name=f"cd")
                    if t == 0:
                        nc.vector.tensor_scalar_mul(out=c_new[d][:], in0=u[:], scalar1=2.0)
                    else:
                        t1 = rp.tile([128, KCH, GB], F32, tag=f"t1{d}", name=f"t1d")
                        nc.vector.tensor_tensor(
                            out=t1[:], in0=sg[d][:, 3:6, :], in1=c_prev[d][:], op=ALU.mult
                        )
                        nc.vector.scalar_tensor_tensor(
                            out=c_new[d][:],
                            in0=u[:],
                            scalar=2.0,
                            in1=t1[:],
                            op0=ALU.mult,
                            op1=ALU.add,
                        )
                # --- ACT: tanh(c) ---
                tcs = [None, None]
                for d in (0, 1):
                    tcs[d] = rp.tile([128, KCH, GB], F32, tag=f"tc{d}", name=f"tcd")
                    nc.scalar.activation(out=tcs[d][:], in_=c_new[d][:], func=AF.Tanh)
                # --- DVE: h = sigmoid(o) * tanh(c) ---
                for d in (0, 1):
                    nc.vector.tensor_tensor(
                        out=hv[d][:, :, :, tcol(d, t)],
                        in0=sg[d][:, 9:12, :],
                        in1=tcs[d][:],
                        op=ALU.mult,
                    )
                    c_prev[d] = c_new[d]
                    ps_cur[d] = ps_nxt[d]
                if stream is not None:
                    for d in (0, 1):
                        stream_pending.append((d, tcol(d, t)))
            if stream is not None:
                for d, col, sp in sps_done:
                    emit_stream_move(d, col, sp)
                for d, col in stream_pending:
                    emit_stream_move(d, col, emit_stream_mm(d, col))

    # ---- layer pipeline with scoped lifetimes (strict LIFO pools) ----
    with tc.tile_pool(name="phh", bufs=1) as phh:
        h0f = phh.tile([128, KCH, NT], BF16, name="h0f")
        h0b = phh.tile([128, KCH, NT], BF16, name="h0b")
        h1f = phh.tile([128, KCH, NT], BF16, name="h1f")
        h1b = phh.tile([128, KCH, NT], BF16, name="h1b")
        with tc.tile_pool(name="pg", bufs=1) as pgp:
            G_sb = pgp.tile([128, 2 * MCH, NT], BF16, name="G_sb")
            with tc.tile_pool(name="pw0", bufs=1) as pw0:
                wih0 = pw0.tile([128, DCH, 2 * 4 * HD], BF16, name="wih0")
                nc.sync.dma_start(
                    out=wih0[:], in_=ins["wih0T"].rearrange("(k p) m -> p k m", p=128)
                )
                with tc.tile_pool(name="px", bufs=1) as px:
                    posty_sb = px.tile([128, 2, D], F32, name="posty_sb")
                    nc.sync.dma_start(
                        out=posty_sb[:],
                        in_=ins["posty"].rearrange("(a p) d -> p a d", p=128),
                    )
                    dve_touch(posty_sb[0:1, 0, 0:1])
                    xT_a = px.tile([128, DCH, 512], BF16, name="xT_a")
                    xT_b = px.tile([128, DCH, 512], BF16, name="xT_b")
                    s1_embed(xT_a, range(0, 4), posty_sb)
                    g_matmul(
                        0, G_sb, lambda kk, nb: xT_a[:, kk, :], wih0, nbs=[0], tag="a"
                    )
                    s1_embed(xT_b, range(4, NTILE), posty_sb)
                    g_matmul(
                        0, G_sb, lambda kk, nb: xT_b[:, kk, :], wih0, nbs=[1], tag="b"
                    )
            with tc.tile_pool(name="prec", bufs=1) as prec:
                # loads have no deps, so their DMAs overlap the G0 matmuls
                whh0 = prec.tile([128, KCH, 2 * 4 * HD], BF16, name="whh0")
                nc.sync.dma_start(
                    out=whh0[:], in_=ins["whh0T"].rearrange("(k p) m -> p k m", p=128)
                )
                whh1 = prec.tile([128, KCH, 2 * 4 * HD], BF16, name="whh1")
                nc.sync.dma_start(
                    out=whh1[:], in_=ins["whh1T"].rearrange("(k p) m -> p k m", p=128)
                )
                wih1 = prec.tile([128, DCH, 2 * 4 * HD], BF16, name="wih1")
                nc.sync.dma_start(
                    out=wih1[:], in_=ins["wih1T"].rearrange("(k p) m -> p k m", p=128)
                )
                G1_sb = prec.tile([128, 2 * MCH, NT], BF16, name="G1_sb")
                b1rep = prec.tile([128, 2 * MCH, GB], BF16, name="b1rep")
                for bq in range(GB):
                    nc.vector.tensor_copy(
                        out=b1rep[:, :, bq : bq + 1],
                        in_=b_sb[:, 2 * MCH : 4 * MCH].rearrange("p (m o) -> p m o", o=1),
                    )
                if DEBUG_OUTS:
                    nc.sync.dma_start(out=dbg["dbg_g"], in_=G_sb[:])
                # fold the DVE- and ACT-written G halves into PE's clock so
                # the recurrence's injects/matmuls carry one sem wait each.
                pe_touch(G_sb[:, 0, 0:1])
                pe_touch(G_sb[:, 0, 512:513])
                with tc.tile_pool(name="r0st", bufs=1, space="PSUM") as spool:
                    recurrence(
                        0, G_sb, h0f, h0b, whh0, stream=(G1_sb, wih1, spool, b1rep)
                    )
                if DEBUG_OUTS:
                    nc.sync.dma_start(out=dbg["dbg_h0f"], in_=h0f[:])
                    nc.sync.dma_start(out=dbg["dbg_h0b"], in_=h0b[:])
                pe_touch(G1_sb[:, 0, 0:1])
                recurrence(1, G1_sb, h1f, h1b, whh1)
        if DEBUG_OUTS:
            nc.sync.dma_start(out=dbg["dbg_h1f"], in_=h1f[:])
            nc.sync.dma_start(out=dbg["dbg_h1b"], in_=h1b[:])

        # ---- emissions: em^T [C, NT] = fc @ concat(h1f, h1b) + fc_b ----
        crf_cm = tc.tile_pool(name="crf", bufs=1)
        crf = crf_cm.__enter__()
        labf_sb = crf.tile([1, NT], F32, name="labf_sb")
        nc.sync.dma_start(out=labf_sb[:], in_=ins["labf"])
        pe_touch(cpack_sb[:, 0:1])
        tileA = crf.tile([C, NT], F32, name="tileA")  # emT, later M1/pd
        tileB = crf.tile([C, NT], F32, name="tileB")  # Q
        tileC = crf.tile([C, NT], F32, name="tileC")  # lab_bc, later gem
        tileD = crf.tile([C, NT], F32, name="tileD")  # OH
        emT = tileA
        with tc.tile_pool(name="emps", bufs=2, space="PSUM") as emps:
            nc.tensor.ldweights(weights=fcT_sb[:, 0, 0:1])
            for nb in range(NT // 512):
                ps = emps.tile([128, 512], F32, tag="emps")
                for kk in range(DCH):
                    src = h1f if kk < KCH else h1b
                    nc.tensor.matmul(
                        out=ps[:C, :],
                        lhsT=fcT_sb[:, kk, :],
                        rhs=src[:, kk % KCH, 512 * nb : 512 * (nb + 1)],
                        start=(kk == 0),
                        stop=(kk == DCH - 1),
                    )
                nc.vector.tensor_scalar_add(
                    out=emT[:, 512 * nb : 512 * (nb + 1)], in0=ps[:C, :], scalar1=fcb_sb[:]
                )
        if DEBUG_OUTS:
            nc.sync.dma_start(out=dbg["dbg_em"], in_=emT[:])

        # ---- CRF ----
        with tc.tile_pool(name="crfw", bufs=4) as cw, tc.tile_pool(
            name="crfps", bufs=1, space="PSUM"
        ) as cps:
            Q = tileB
            nc.scalar.activation(out=Q[:], in_=emT[:], func=AF.Exp)
            dve_touch(Q[0:1, 0:1])
            Qv = Q[:].rearrange("c (b t) -> c b t", b=GB)

            # Bidirectional scan in exp domain, meeting at s = T/2 - 1:
            #   alpha:  v_t = (E^T v_{t-1}) * q_t,        t = 1..s
            #   beta:   b_{t-1} = E (q_t * b_t),          t = T-1..s+1
            #   Z = sum_i v_s[i] * b_s[i]
            # Renorm is LAZY: the 1/s scale from a renorm is folded into
            # that chain's q a few steps in its own future (off the serial
            # chain); ln(s) values are batch-processed at the end.
            LAG = 3
            NREN = 32
            TMID = T // 2          # meet at s = TMID - 1
            s_store = cw.tile([1, GB, NREN], F32, tag="s_store")
            nc.vector.memset(s_store[:], 1.0)
            v_prev = cw.tile([C, GB], F32, tag="v")
            nc.vector.tensor_scalar_mul(out=v_prev[:], in0=Qv[:, :, 0], scalar1=expst_sb[:])
            b_ps = None            # beta state lives in PSUM between steps
            qs_a = {}
            qs_b = {}
            nren_a, nren_b = 0, 0

            def renorm(chain_rhs_sb, qcol, pend, slot):
                sps = cps.tile([1, GB], F32, tag="cps1", bufs=1, name="sps")
                nc.tensor.matmul(
                    out=sps[:], lhsT=onesC1[:], rhs=chain_rhs_sb, start=True, stop=True
                )
                nc.vector.tensor_copy(out=s_store[:, :, slot], in_=sps[:])
                rv = cw.tile([1, GB], F32, tag="rv", name="rv")
                nc.vector.reciprocal(out=rv[:], in_=s_store[:, :, slot])
                rvb = cps.tile([C, GB], F32, tag="rvb", bufs=1, name="rvb")
                nc.tensor.matmul(out=rvb[:], lhsT=ones1C[:], rhs=rv[:], start=True, stop=True)
                qs = cw.tile([C, GB], F32, tag="qs", bufs=4, name="qs")
                nc.vector.tensor_tensor(out=qs[:], in0=rvb[:], in1=Qv[:, :, qcol], op=ALU.mult)
                pend[qcol] = qs

            # --- score-prep as spaced tasks run inside the scan's idle
            # windows (big ops split in half to bound queue-head stalls) ---
            lab_bc = tileC
            OH = tileD
            gem = tileC
            M1 = tileA
            gem_r2 = cw.tile([C, 2], F32, tag="gred2")
            pd_r2 = cw.tile([C, 2], F32, tag="pdr2")
            st_r = cw.tile([C, 1], F32, tag="str")
            en_r = cw.tile([C, 1], F32, tag="enr")
            score_sb = cw.tile([1, 1], F32, tag="scoresb")
            OHv = OH[:].rearrange("c (b t) -> c b t", b=GB)
            pdv = M1[:].rearrange("c (b t) -> c b t", b=GB)

            def tk_lab(nb):
                bps = cps.tile([C, 512], F32, tag="cps512", name="bps")
                nc.tensor.matmul(
                    out=bps[:],
                    lhsT=ones1C[:],
                    rhs=labf_sb[:, 512 * nb : 512 * (nb + 1)],
                    start=True,
                    stop=True,
                )
                nc.vector.tensor_copy(out=lab_bc[:, 512 * nb : 512 * (nb + 1)], in_=bps[:])

            def tk_oh(nb):
                nc.vector.tensor_scalar(
                    out=OH[:, 512 * nb : 512 * (nb + 1)],
                    in0=lab_bc[:, 512 * nb : 512 * (nb + 1)],
                    scalar1=iota_sb[:],
                    scalar2=None,
                    op0=ALU.is_equal,
                )

            def tk_gem(nb):
                nc.vector.tensor_tensor(
                    out=gem[:, 512 * nb : 512 * (nb + 1)],
                    in0=emT[:, 512 * nb : 512 * (nb + 1)],
                    in1=OH[:, 512 * nb : 512 * (nb + 1)],
                    op=ALU.mult,
                )

            def tk_gem_r(nb):
                nc.vector.reduce_sum(
                    out=gem_r2[:, nb : nb + 1],
                    in_=gem[:, 512 * nb : 512 * (nb + 1)],
                    axis=mybir.AxisListType.X,
                )

            def tk_m1(nb):
                lo = 512 * nb
                hi = min(512 * (nb + 1), NT - 1)
                mps = cps.tile([C, 512], F32, tag="cps512", name="mps")
                nc.tensor.matmul(
                    out=mps[:, : hi - lo],
                    lhsT=transT_sb[:],
                    rhs=OH[:, lo + 1 : hi + 1],
                    start=True,
                    stop=True,
                )
                nc.vector.tensor_copy(out=M1[:, lo:hi], in_=mps[:, : hi - lo])

            def tk_m1mult(nb):
                lo = 512 * nb
                hi = min(512 * (nb + 1), NT - 1)
                nc.vector.tensor_tensor(
                    out=M1[:, lo:hi], in0=OH[:, lo:hi], in1=M1[:, lo:hi], op=ALU.mult
                )

            def tk_pd(half):
                nc.vector.reduce_sum(
                    out=pd_r2[:, half : half + 1],
                    in_=pdv[:, 2 * half : 2 * half + 2, 0 : T - 1],
                    axis=mybir.AxisListType.XY,
                )

            def tk_sten():
                st8 = cw.tile([C, GB], F32, tag="st8", name="st8")
                nc.vector.tensor_scalar_mul(out=st8[:], in0=OHv[:, :, 0], scalar1=stv_sb[:])
                nc.vector.reduce_sum(out=st_r[:], in_=st8[:], axis=mybir.AxisListType.X)
                en8 = cw.tile([C, GB], F32, tag="en8", name="en8")
                nc.vector.tensor_scalar_mul(out=en8[:], in0=OHv[:, :, T - 1], scalar1=env_sb[:])
                nc.vector.reduce_sum(out=en_r[:], in_=en8[:], axis=mybir.AxisListType.X)

            def tk_score():
                score_ps = pscr  # reuse the absorber PSUM bank
                parts = (
                    gem_r2[:, 0:1], gem_r2[:, 1:2],
                    pd_r2[:, 0:1], pd_r2[:, 1:2],
                    st_r[:], en_r[:],
                )
                for q, r in enumerate(parts):
                    nc.tensor.matmul(
                        out=score_ps[:1, :1],
                        lhsT=onesC1[:],
                        rhs=r,
                        start=(q == 0),
                        stop=(q == len(parts) - 1),
                        skip_group_check=True,
                    )
                nc.vector.tensor_copy(out=score_sb[:], in_=score_ps[:1, :1])

            tasks = [
                lambda: tk_lab(0), lambda: tk_lab(1),
                lambda: tk_oh(0), lambda: tk_oh(1),
                lambda: tk_gem(0), lambda: tk_gem(1),
                lambda: tk_gem_r(0), lambda: tk_gem_r(1),
                lambda: tk_m1(0), lambda: tk_m1(1),
                lambda: tk_m1mult(0), lambda: tk_m1mult(1),
                lambda: tk_pd(0), lambda: tk_pd(1),
                tk_sten, tk_score,
            ]

            for i in range(TMID):
                if tasks and i % 4 == 1:
                    tasks.pop(0)()
                # --- alpha step t = i+1 (runs for i = 0..TMID-2) ---
                ta = i + 1
                if ta <= TMID - 1:
                    vps = cps.tile([C, GB], F32, tag="vps", bufs=2, name="vps")
                    nc.tensor.matmul(
                        out=vps[:], lhsT=E_sb[:], rhs=v_prev[:], start=True, stop=True
                    )
                    v_new = cw.tile([C, GB], F32, tag="v", name="v_new")
                    qt = qs_a.pop(ta, None)
                    nc.vector.tensor_tensor(
                        out=v_new[:],
                        in0=vps[:],
                        in1=(qt[:] if qt is not None else Qv[:, :, ta]),
                        op=ALU.mult,
                    )
                    v_prev = v_new
                    if ta % RENORM == RENORM - 1 and ta + LAG <= TMID - 1:
                        renorm(v_prev[:], ta + LAG, qs_a, nren_a)
                        nren_a += 1
                # --- beta step t = T-1-i: w = q_t * b_t ; b_{t-1} = E w ---
                tb = T - 1 - i
                w = cw.tile([C, GB], F32, tag="w", name="w")
                qt = qs_b.pop(tb, None)
                qin = qt[:] if qt is not None else Qv[:, :, tb]
                if b_ps is None:
                    nc.vector.tensor_scalar_mul(out=w[:], in0=qin, scalar1=expen_sb[:])
                else:
                    nc.vector.tensor_tensor(out=w[:], in0=qin, in1=b_ps[:], op=ALU.mult)
                b_ps = cps.tile([C, GB], F32, tag="bps", bufs=2, name="b_ps")
                nc.tensor.matmul(out=b_ps[:], lhsT=expTT_sb[:], rhs=w[:], start=True, stop=True)
                j = i + 1  # beta steps completed
                if j % RENORM == 3 and i + 1 + LAG < TMID:
                    renorm(w[:], tb - 1 - LAG, qs_b, 16 + nren_b)
                    nren_b += 1

            # Z = colsum(v_mid * b_mid)
            zv = cw.tile([C, GB], F32, tag="zv")
            nc.vector.tensor_tensor(out=zv[:], in0=v_prev[:], in1=b_ps[:], op=ALU.mult)
            zps = cps.tile([1, GB], F32, tag="cps1", bufs=1)
            nc.tensor.matmul(out=zps[:], lhsT=onesC1[:], rhs=zv[:], start=True, stop=True)
            lnz = cw.tile([1, GB], F32, tag="lnz")
            nc.scalar.activation(out=lnz[:], in_=zps[:], func=AF.Ln)
            lns_all = cw.tile([1, GB, NREN], F32, tag="lns_all")
            nc.scalar.activation(out=lns_all[:], in_=s_store[:], func=AF.Ln)
            off = cw.tile([1, GB], F32, tag="off")
            nc.vector.reduce_sum(out=off[:], in_=lns_all[:], axis=mybir.AxisListType.X)
            logz = cw.tile([1, GB], F32, tag="logz")
            nc.vector.tensor_tensor(out=logz[:], in0=lnz[:], in1=off[:], op=ALU.add)
            lz_tot = cw.tile([1, 1], F32, tag="lztot")
            nc.vector.reduce_sum(out=lz_tot[:], in_=logz[:], axis=mybir.AxisListType.X)
            loss_sb = cw.tile([1, 1], F32, tag="loss_sb")
            nc.vector.tensor_tensor(out=loss_sb[:], in0=lz_tot[:], in1=score_sb[:], op=ALU.subtract)
            nc.sync.dma_start(out=loss_out, in_=loss_sb[:])
            if DEBUG_OUTS:
                dsc = cw.tile([1, 2], F32, tag="dsc")
                nc.vector.tensor_copy(out=dsc[:, 0:1], in_=lz_tot[:])
                nc.vector.tensor_copy(out=dsc[:, 1:2], in_=score_sb[:])
                nc.sync.dma_start(out=dbg["dbg_sc"], in_=dsc[:])
        crf_cm.__exit__(None, None, None)

    est.close()


# ---------------------------------------------------------------------------
# host side
# ---------------------------------------------------------------------------

def make_in_maps(inputs):
    ids = np.asarray(inputs["input_ids"]).astype(np.int64)
    labels = np.asarray(inputs["labels"]).astype(np.int64)
    word_emb = _f32(inputs["word_emb"])
    pos_emb = _f32(inputs["pos_emb"])
    type_emb = _f32(inputs["type_emb"])
    ln_g = _f32(inputs["ln_g"])
    ln_b = _f32(inputs["ln_b"])
    w_ih = _f32(inputs["w_ih"])
    w_hh = _f32(inputs["w_hh"])
    b_ih = _f32(inputs["b_ih"])
    b_hh = _f32(inputs["b_hh"])
    fc_w = _f32(inputs["fc_w"])
    fc_b = _f32(inputs["fc_b"])
    crf_start = _f32(inputs["crf_start"])
    crf_end = _f32(inputs["crf_end"])
    crf_trans = _f32(inputs["crf_trans"])

    posty = np.ascontiguousarray(pos_emb[:T] + type_emb[0][None, :])

    def sig_trick(w, b):
        # fold tanh(x) = 2*sigmoid(2x)-1: scale g-gate rows (2HD:3HD) by 2
        w = w.copy()
        b = b.copy()
        w[2 * HD : 3 * HD] *= 2.0
        b[2 * HD : 3 * HD] *= 2.0
        return w, b

    # per-layer packed weights, both directions: cols [fwd 4HD | bwd 4HD]
    wihT = []
    whhT = []
    biases = []  # [l][d] -> (1536,)
    for l in range(L):
        wl = []
        hl = []
        bl = []
        for d in range(2):
            w = w_ih[l, d]
            bias = b_ih[l, d] + b_hh[l, d]
            if l == 0:
                bias = bias + w @ ln_b
                w = w * ln_g[None, :]
            u = w_hh[l, d]
            w, bias = sig_trick(w, bias)
            u, _ = sig_trick(u, np.zeros(4 * HD, np.float32))
            wl.append(w.T)   # [in_dim, 1536]
            hl.append(u.T)   # [HD, 1536]
            bl.append(bias)
        wihT.append(np.concatenate(wl, axis=1))   # [in_dim, 3072]
        whhT.append(np.concatenate(hl, axis=1))   # [HD, 3072]
        biases.append(bl)

    # b01 [128, 48]: col l*24 + d*12 + m holds bias[l][d][128m:128(m+1)]
    bcols = []
    for l in range(L):
        for d in range(2):
            bcols.append(biases[l][d].reshape(MCH, 128).T)
    b01 = np.ascontiguousarray(np.concatenate(bcols, axis=1))

    cpack = np.zeros((C, 48), np.float32)
    cpack[:, 0:C] = np.exp(crf_trans)
    cpack[:, C : 2 * C] = crf_trans.T
    cpack[:, 28] = np.exp(crf_start)
    cpack[:, 29] = np.exp(crf_end)
    cpack[:, 30] = crf_start
    cpack[:, 31] = crf_end
    cpack[:, 32] = np.arange(C, dtype=np.float32)
    cpack[:, 33] = fc_b
    cpack[:, 34:48] = np.exp(crf_trans).T

    shared = dict(
        word_emb=word_emb,
        posty=posty,
        wih0T=_bf(wihT[0]),
        wih1T=_bf(wihT[1]),
        whh0T=_bf(whhT[0]),
        whh1T=_bf(whhT[1]),
        b01=b01,
        fcT=_bf(fc_w.T),
        cpack=cpack,
    )

    in_maps = []
    for core in range(NCORES):
        sl = slice(GB * core, GB * (core + 1))
        in_maps.append(
            dict(
                ids32=np.ascontiguousarray(ids[sl].reshape(NT, 1).astype(np.int32)),
                labf=np.ascontiguousarray(labels[sl].reshape(1, NT).astype(np.float32)),
                **shared,
            )
        )
    return in_maps


_PROGRAM = None
_COST_MODEL_NS = None


def _get_program():
    global _PROGRAM, _COST_MODEL_NS
    if _PROGRAM is None:
        _PROGRAM = build_program()
        try:
            from concourse.timeline_sim import TimelineSim

            _COST_MODEL_NS = int(TimelineSim(_PROGRAM, trace=False, no_exec=True).simulate())
        except Exception:
            _COST_MODEL_NS = None
    return _PROGRAM


def run(inputs, trace=False):
    nc = _get_program()
    in_maps = make_in_maps(inputs)
    res = run_bass_kernel_spmd(nc, in_maps, core_ids=list(range(NCORES)), trace=trace)
    total = np.float64(0.0)
    for c in range(NCORES):
        total += np.float64(res.results[c]["loss"][0, 0])
    return np.asarray(total, dtype=np.float32), res


def kernel(**inputs):
    out, _ = run(inputs, trace=False)
    return out


# revision 61
# speedup vs baseline: 1.0037x; 1.0037x over previous
"""BiLSTM-CRF sequence-tagging loss on 8 Trainium2 NeuronCores.

Sharding: pure data-parallel — core c owns sequences [4c, 4c+4) and runs
BOTH LSTM directions locally (no collectives at all).  The backward
direction writes its h-stream time-reversed directly (free AP offset), so
layer-1/emission inputs are plain [h_fwd | h_bwd] concats.

Per-step recurrence structure (per direction, chains interleaved so the
two directions hide each other's latency):
  PE   : G+bias injected into PSUM via identity matmul (prefetched one
         step ahead into the alternate bank) + 36 Whh matmuls.
  ACT  : one Sigmoid over all 12 gate chunks — the g-gate uses
         tanh(x) = 2*sigmoid(2x) - 1 with the 2x folded into the weights.
  DVE  : u = (sg - 0.5) * si ; t1 = sf * c_prev ; c = 2u + t1 (fused
         scalar_tensor_tensor ops).
  ACT  : tc = tanh(c)
  DVE  : h = so * tc  (written straight into the bf16 h stream).
"""

import os
import sys

import numpy as np

for _p in ("/opt/trn_rl_repo", "/root/.axon_site/_ro/trn_rl_repo"):
    if os.path.isdir(_p) and _p not in sys.path:
        sys.path.insert(0, _p)

import ml_dtypes  # noqa: E402

import concourse.bass as bass  # noqa: E402
import concourse.bacc as bacc  # noqa: E402
import concourse.tile as tile  # noqa: E402
from concourse import mybir  # noqa: E402
from concourse.bass import IndirectOffsetOnAxis  # noqa: E402
from concourse.bass_utils import run_bass_kernel_spmd  # noqa: E402
from concourse.masks import make_identity  # noqa: E402

F32 = mybir.dt.float32
BF16 = mybir.dt.bfloat16
I32 = mybir.dt.int32
AF = mybir.ActivationFunctionType
ALU = mybir.AluOpType

# problem shapes (hardcoded per contract)
B, T, V, D, C, HD = 32, 256, 30522, 768, 14, 384
L = 2
NCORES = 8
GB = 4             # sequences per core
NT = GB * T        # tokens per core = 1024
NTILE = NT // 128  # 8
MCH = 12           # gate chunks of 128 per direction (4*HD/128)
KCH = 3            # hidden chunks per direction (HD/128)
DCH = 6            # input-dim chunks (D/128)
LN_EPS = 1e-12
RENORM = 8

DEBUG_OUTS = False


def _bf(x):
    return np.ascontiguousarray(np.asarray(x, dtype=np.float32)).astype(ml_dtypes.bfloat16)


def _f32(x):
    return np.ascontiguousarray(np.asarray(x, dtype=np.float32))


# ---------------------------------------------------------------------------
# device program
# ---------------------------------------------------------------------------

def build_program():
    nc = bacc.Bacc("TRN2", target_bir_lowering=False, debug=False, num_devices=NCORES)

    def din(name, shape, dt):
        return nc.dram_tensor(name, shape, dt, kind="ExternalInput").ap()

    ins = dict(
        ids32=din("ids32", [NT, 1], I32),
        labf=din("labf", [1, NT], F32),
        word_emb=din("word_emb", [V, D], F32),
        posty=din("posty", [T, D], F32),
        wih0T=din("wih0T", [D, 2 * 4 * HD], BF16),
        wih1T=din("wih1T", [D, 2 * 4 * HD], BF16),
        whh0T=din("whh0T", [HD, 2 * 4 * HD], BF16),
        whh1T=din("whh1T", [HD, 2 * 4 * HD], BF16),
        b01=din("b01", [128, 4 * MCH], F32),
        fcT=din("fcT", [D, C], BF16),
        cpack=din("cpack", [C, 48], F32),
    )

    loss_out = nc.dram_tensor("loss", [1, 1], F32, kind="ExternalOutput").ap()
    dbg = {}
    if DEBUG_OUTS:
        dbg["dbg_xt"] = nc.dram_tensor("dbg_xt", [128, DCH, NT], BF16, kind="ExternalOutput").ap()
        dbg["dbg_g"] = nc.dram_tensor("dbg_g", [128, 2 * MCH, NT], BF16, kind="ExternalOutput").ap()
        dbg["dbg_h0f"] = nc.dram_tensor("dbg_h0f", [128, KCH, NT], BF16, kind="ExternalOutput").ap()
        dbg["dbg_h0b"] = nc.dram_tensor("dbg_h0b", [128, KCH, NT], BF16, kind="ExternalOutput").ap()
        dbg["dbg_h1f"] = nc.dram_tensor("dbg_h1f", [128, KCH, NT], BF16, kind="ExternalOutput").ap()
        dbg["dbg_h1b"] = nc.dram_tensor("dbg_h1b", [128, KCH, NT], BF16, kind="ExternalOutput").ap()
        dbg["dbg_em"] = nc.dram_tensor("dbg_em", [C, NT], F32, kind="ExternalOutput").ap()
        dbg["dbg_sc"] = nc.dram_tensor("dbg_sc", [1, 2], F32, kind="ExternalOutput").ap()

    with tile.TileContext(nc) as tc:
        _build_body(tc, ins, loss_out, dbg)

    nc.compile()
    return nc


def _build_body(tc, ins, loss_out, dbg):
    nc = tc.nc
    from contextlib import ExitStack

    est = ExitStack()
    pers = est.enter_context(tc.tile_pool(name="pers", bufs=1))

    # scratch + absorbers: only ONE sem wait per instruction is allowed, so
    # junctions of two producers get a tiny absorber op that folds one
    # producer into the consuming engine's clock first.
    scr_dve = pers.tile([1, 4], F32, name="scr_dve")
    scr_act = pers.tile([1, 4], F32, name="scr_act")
    pabs = est.enter_context(tc.tile_pool(name="pabs", bufs=1, space="PSUM"))
    pscr = pabs.tile([1, 8], F32, name="pscr")

    scr_gp = pers.tile([1, 4], F32, name="scr_gp")

    def dve_touch(ap):
        nc.vector.tensor_copy(out=scr_dve[:, 0:1], in_=ap)

    def gp_touch(ap):
        nc.gpsimd.tensor_copy(out=scr_gp[:, 0:1], in_=ap)

    def act_touch(ap):
        nc.scalar.activation(out=scr_act[:, 0:1], in_=ap, func=AF.Copy)

    def pe_touch(ap_col):
        nc.tensor.matmul(out=pscr[:1, :1], lhsT=ap_col, rhs=ap_col, start=True, stop=True)

    b_sb = pers.tile([128, 4 * MCH], F32, name="b_sb")
    nc.sync.dma_start(out=b_sb[:], in_=ins["b01"])
    dve_touch(b_sb[0:1, 0:1])
    act_touch(b_sb[0:1, 0:1])

    fcT_sb = pers.tile([128, DCH, C], BF16, name="fcT")
    nc.sync.dma_start(out=fcT_sb[:], in_=ins["fcT"].rearrange("(k p) m -> p k m", p=128))

    cpack_sb = pers.tile([C, 48], F32, name="cpack_sb")
    nc.sync.dma_start(out=cpack_sb[:], in_=ins["cpack"])
    dve_touch(cpack_sb[0:1, 0:1])
    E_sb = cpack_sb[:, 0:C]
    transT_sb = cpack_sb[:, C : 2 * C]
    expst_sb = cpack_sb[:, 28:29]
    expen_sb = cpack_sb[:, 29:30]
    stv_sb = cpack_sb[:, 30:31]
    env_sb = cpack_sb[:, 31:32]
    iota_sb = cpack_sb[:, 32:33]
    fcb_sb = cpack_sb[:, 33:34]
    expTT_sb = cpack_sb[:, 34:48]

    ids_sb = pers.tile([128, NTILE], I32, name="ids_sb")
    nc.sync.dma_start(out=ids_sb[:], in_=ins["ids32"].rearrange("(k p) o -> p (k o)", p=128))

    ident = pers.tile([128, 128], F32, name="ident")
    make_identity(nc, ident[:])
    pe_touch(ident[:, 0:1])
    identb = pers.tile([128, 128], BF16, name="identb")
    make_identity(nc, identb[:])
    pe_touch(identb[:, 0:1])
    eps_sb = pers.tile([128, 1], F32, name="eps_sb")
    nc.vector.memset(eps_sb[:], LN_EPS)
    ones1C = pers.tile([1, C], F32, name="ones1C")
    nc.vector.memset(ones1C[:], 1.0)
    onesC1 = pers.tile([C, 1], F32, name="onesC1")
    nc.vector.memset(onesC1[:], 1.0)

    # ---- helpers ----
    def s1_embed(xT_sb, ks, posty_sb):
        with tc.tile_pool(name="s1", bufs=3) as s1, tc.tile_pool(
            name="s1ps", bufs=4, space="PSUM"
        ) as s1ps:
            for k in ks:
                emb = s1.tile([128, D], F32, tag="emb")
                nc.gpsimd.indirect_dma_start(
                    out=emb[:],
                    out_offset=None,
                    in_=ins["word_emb"],
                    in_offset=IndirectOffsetOnAxis(ap=ids_sb[:, k : k + 1], axis=0),
                )
                nc.vector.tensor_add(out=emb[:], in0=emb[:], in1=posty_sb[:, k % 2, :])
                stats = s1.tile([128, 3, 6], F32, tag="stats")
                embv = emb[:].rearrange("p (s q) -> p s q", s=3)
                for sg_ in range(3):
                    nc.vector.bn_stats(out=stats[:, sg_, :], in_=embv[:, sg_, :])
                mv = s1.tile([128, 2], F32, tag="mv")
                nc.vector.bn_aggr(out=mv[:], in_=stats[:])
                std = s1.tile([128, 1], F32, tag="std")
                nc.scalar.activation(out=std[:], in_=mv[:, 1:2], func=AF.Sqrt, bias=eps_sb[:])
                rstd = s1.tile([128, 1], F32, tag="rstd")
                nc.vector.reciprocal(out=rstd[:], in_=std[:])
                xln = s1.tile([128, D], F32, tag="xln")
                nc.vector.tensor_scalar(
                    out=xln[:],
                    in0=emb[:],
                    scalar1=mv[:, 0:1],
                    scalar2=rstd[:],
                    op0=ALU.subtract,
                    op1=ALU.mult,
                )
                kl = k - ks[0]
                for j in range(DCH):
                    tp = s1ps.tile([128, 128], F32, tag="tp")
                    nc.tensor.transpose(
                        out=tp[:], in_=xln[:, 128 * j : 128 * (j + 1)], identity=ident[:]
                    )
                    if j % 2 == 0:
                        nc.vector.tensor_copy(
                            out=xT_sb[:, j, 128 * kl : 128 * (kl + 1)], in_=tp[:]
                        )
                    else:
                        nc.scalar.activation(
                            out=xT_sb[:, j, 128 * kl : 128 * (kl + 1)], in_=tp[:], func=AF.Copy
                        )

    def g_matmul(l, G_sb, rhs_of, wih, nbs=None, tag="", ncol=512):
        """G = x @ Wih^T + bias for both dirs, written into G_sb (bf16)."""
        with tc.tile_pool(name=f"g{l}ps{tag}", bufs=4, space="PSUM") as gps:
            nc.tensor.ldweights(weights=wih[:, 0, 0:1])
            for m in range(2 * MCH):
                for nb in (range(NT // ncol) if nbs is None else nbs):
                    ps = gps.tile([128, ncol], F32, tag="gps", name="gps")
                    for kk in range(DCH):
                        nc.tensor.matmul(
                            out=ps[:],
                            lhsT=wih[:, kk, 128 * m : 128 * (m + 1)],
                            rhs=rhs_of(kk, nb),
                            start=(kk == 0),
                            stop=(kk == DCH - 1),
                        )
                    bcol = b_sb[:, l * 2 * MCH + m : l * 2 * MCH + m + 1]
                    if (m + nb) % 2 == 0:
                        nc.vector.tensor_scalar_add(
                            out=G_sb[:, m, ncol * nb : ncol * (nb + 1)],
                            in0=ps[:],
                            scalar1=bcol,
                        )
                    else:
                        nc.scalar.activation(
                            out=G_sb[:, m, ncol * nb : ncol * (nb + 1)],
                            in_=ps[:],
                            func=AF.Identity,
                            bias=bcol,
                        )

    def recurrence(l, G_sb, hf, hb, whh, bias_rep=None, stream=None):
        """Both-direction LSTM over local time, staggered chains.

        Direction d reads G chunks [12d:12d+12] at time col (fwd: t, bwd:
        T-1-t) and writes its h at the same time col -> both h streams end
        up time-ordered.

        bias_rep: [128, 24, GB] bias tile injected into PSUM with G (used
        when G_sb itself carries no bias).
        stream: (G1_sb, wih1, spool) — accumulate the NEXT layer's G into
        G1_sb column-by-column as h values appear (fills PE/DVE idle)."""
        Gv = G_sb[:].rearrange("p m (b t) -> p m b t", b=GB)
        hv = [h[:].rearrange("p c (b t) -> p c b t", b=GB) for h in (hf, hb)]
        if stream is not None:
            G1_sb, wih1s, spool, bias1s = stream
            G1v = G1_sb[:].rearrange("p m (b t) -> p m b t", b=GB)
            nc.tensor.ldweights(weights=wih1s[:, 0, 0:1])

        with tc.tile_pool(name=f"r{l}", bufs=3) as rp, tc.tile_pool(
            name=f"r{l}c", bufs=2
        ) as rcp, tc.tile_pool(
            name=f"r{l}psA", bufs=3, space="PSUM"
        ) as rpsA, tc.tile_pool(
            name=f"r{l}psB", bufs=2, space="PSUM"
        ) as rpsB:
            rps = [rpsA, rpsB]
            ps_cur = [None, None]   # PSUM tile holding this step's preacts
            ps_nxt = [None, None]
            c_prev = [None, None]
            nc.tensor.ldweights(weights=whh[:, 0, 0:1])

            stream_pending = []
            sps_done = []

            def tcol(d, t):
                return t if d == 0 else T - 1 - t

            def emit_stream_mm(d, col):
                sp = spool.tile([128, 2 * MCH, GB], F32, tag=f"sp{d}", name="sp")
                first = (col <= T // 2 - 1) if d == 0 else (col >= T // 2)
                # open the accumulation with the bias (first touch) or the
                # column's current partial sum (second touch), so the final
                # move is a plain copy on either engine.
                nc.tensor.matmul(
                    out=sp[:],
                    lhsT=identb[:],
                    rhs=(bias1s[:] if first else G1v[:, :, :, col]),
                    start=True,
                    stop=False,
                    skip_group_check=True,
                )
                for m in range(2 * MCH):
                    for kj in range(KCH):
                        nc.tensor.matmul(
                            out=sp[:, m, :],
                            lhsT=wih1s[:, kj + KCH * d, 128 * m : 128 * (m + 1)],
                            rhs=hv[d][:, kj, :, col],
                            start=False,
                            stop=(kj == KCH - 1),
                            skip_group_check=True,
                        )
                return sp

            def emit_stream_move(d, col, sp):
                # plain copy, alternated across ACT/DVE by direction so each
                # engine absorbs one move per step in its idle window.
                if d == 0:
                    nc.scalar.activation(out=G1v[:, :, :, col], in_=sp[:], func=AF.Copy)
                else:
                    nc.vector.tensor_copy(out=G1v[:, :, :, col], in_=sp[:])

            def emit_inject(d, tt, close):
                psn = rps[d].tile([128, MCH, GB], F32, tag=f"ps{d}", name=f"psd")
                nc.tensor.matmul(
                    out=psn[:],
                    lhsT=identb[:],
                    rhs=Gv[:, MCH * d : MCH * (d + 1), :, tcol(d, tt)],
                    start=True,
                    stop=(close and bias_rep is None),
                    skip_group_check=True,
                )
                if bias_rep is not None:
                    nc.tensor.matmul(
                        out=psn[:],
                        lhsT=ident[:],
                        rhs=bias_rep[:, MCH * d : MCH * (d + 1), :],
                        start=False,
                        stop=close,
                        skip_group_check=True,
                    )
                return psn

            if bias_rep is not None:
                # t=0 preacts also come via PSUM (G_sb carries no bias)
                for d in (0, 1):
                    ps_cur[d] = emit_inject(d, 0, close=True)

            for t in range(T):
                sg = [None, None]
                # --- flush stream moves whose PSUM inputs are ready; at the
                # top of the cycle both ACT and DVE are idle waiting on the
                # chain, so these are free ---
                if stream is not None:
                    for d, col, sp in sps_done:
                        emit_stream_move(d, col, sp)
                    sps_done = []
                # --- PE: Whh matmuls into the prefetched PSUM bank ---
                if t > 0:
                    for d in (0, 1):
                        ps = ps_cur[d]
                        hprev = hv[d][:, :, :, tcol(d, t - 1)]
                        for kk in range(KCH):
                            for m in range(MCH):
                                nc.tensor.matmul(
                                    out=ps[:, m, :],
                                    lhsT=whh[
                                        :, kk, 1536 * d + 128 * m : 1536 * d + 128 * (m + 1)
                                    ],
                                    rhs=hprev[:, kk, :],
                                    start=False,
                                    stop=(kk == KCH - 1 and m == MCH - 1),
                                    skip_group_check=True,
                                )
                # --- ACT: sigmoid over all 12 gate chunks (both dirs) ---
                for d in (0, 1):
                    sg[d] = rp.tile([128, MCH, GB], F32, tag=f"sg{d}", name=f"sg{d}")
                    if t == 0 and bias_rep is None:
                        src = Gv[:, MCH * d : MCH * (d + 1), :, tcol(d, 0)]
                    else:
                        src = ps_cur[d][:]
                    nc.scalar.activation(out=sg[d][:], in_=src, func=AF.Sigmoid)
                # --- PE: prefetch next step's G+bias into the other bank ---
                if t + 1 < T:
                    for d in (0, 1):
                        ps_nxt[d] = emit_inject(d, t + 1, close=False)
                # --- stream PREVIOUS step's next-layer G column (its h sem
                # is long satisfied, so this runs in PE's idle window after
                # the critical whh burst rather than delaying it) ---
                if stream is not None:
                    for d, col in stream_pending:
                        sps_done.append((d, col, emit_stream_mm(d, col)))
                    stream_pending = []
                # --- DVE: c update ---
                c_new = [None, None]
                for d in (0, 1):
                    u = rp.tile([128, KCH, GB], F32, tag=f"u{d}", name=f"ud")
                    nc.vector.scalar_tensor_tensor(
                        out=u[:],
                        in0=sg[d][:, 6:9, :],
                        scalar=0.5,
                        in1=sg[d][:, 0:3, :],
                        op0=ALU.subtract,
                        op1=ALU.mult,
                    )
                    c_new[d] = rcp.tile([128, KCH, GB], F32, tag=f"c{d}", name=f"cd")
                    if t == 0:
                        nc.vector.tensor_scalar_mul(out=c_new[d][:], in0=u[:], scalar1=2.0)
                    else:
                        t1 = rp.tile([128, KCH, GB], F32, tag=f"t1{d}", name=f"t1d")
                        nc.vector.tensor_tensor(
                            out=t1[:], in0=sg[d][:, 3:6, :], in1=c_prev[d][:], op=ALU.mult
                        )
                        nc.vector.scalar_tensor_tensor(
                            out=c_new[d][:],
                            in0=u[:],
                            scalar=2.0,
                            in1=t1[:],
                            op0=ALU.mult,
                            op1=ALU.add,
                        )
                # --- ACT: tanh(c) ---
                tcs = [None, None]
                for d in (0, 1):
                    tcs[d] = rp.tile([128, KCH, GB], F32, tag=f"tc{d}", name=f"tcd")
                    nc.scalar.activation(out=tcs[d][:], in_=c_new[d][:], func=AF.Tanh)
                # --- DVE: h = sigmoid(o) * tanh(c) ---
                for d in (0, 1):
                    nc.vector.tensor_tensor(
                        out=hv[d][:, :, :, tcol(d, t)],
                        in0=sg[d][:, 9:12, :],
                        in1=tcs[d][:],
                        op=ALU.mult,
                    )
                    c_prev[d] = c_new[d]
                    ps_cur[d] = ps_nxt[d]
                if stream is not None:
                    for d in (0, 1):
                        stream_pending.append((d, tcol(d, t)))
            if stream is not None:
                for d, col, sp in sps_done:
                    emit_stream_move(d, col, sp)
                for d, col in stream_pending:
                    emit_stream_move(d, col, emit_stream_mm(d, col))

    # ---- layer pipeline with scoped lifetimes (strict LIFO pools) ----
    with tc.tile_pool(name="phh", bufs=1) as phh:
        h0f = phh.tile([128, KCH, NT], BF16, name="h0f")
        h0b = phh.tile([128, KCH, NT], BF16, name="h0b")
        h1f = phh.tile([128, KCH, NT], BF16, name="h1f")
        h1b = phh.tile([128, KCH, NT], BF16, name="h1b")
        with tc.tile_pool(name="pg", bufs=1) as pgp:
            G_sb = pgp.tile([128, 2 * MCH, NT], BF16, name="G_sb")
            with tc.tile_pool(name="pw0", bufs=1) as pw0:
                wih0 = pw0.tile([128, DCH, 2 * 4 * HD], BF16, name="wih0")
                nc.sync.dma_start(
                    out=wih0[:], in_=ins["wih0T"].rearrange("(k p) m -> p k m", p=128)
                )
                with tc.tile_pool(name="px", bufs=1) as px:
                    posty_sb = px.tile([128, 2, D], F32, name="posty_sb")
                    nc.sync.dma_start(
                        out=posty_sb[:],
                        in_=ins["posty"].rearrange("(a p) d -> p a d", p=128),
                    )
                    dve_touch(posty_sb[0:1, 0, 0:1])
                    xT_a = px.tile([128, DCH, 512], BF16, name="xT_a")
                    xT_b = px.tile([128, DCH, 512], BF16, name="xT_b")
                    s1_embed(xT_a, range(0, 4), posty_sb)
                    g_matmul(
                        0, G_sb, lambda kk, nb: xT_a[:, kk, :], wih0, nbs=[0], tag="a"
                    )
                    s1_embed(xT_b, range(4, NTILE), posty_sb)
                    g_matmul(
                        0, G_sb, lambda kk, nb: xT_b[:, kk, :], wih0, nbs=[1], tag="b"
                    )
            with tc.tile_pool(name="prec", bufs=1) as prec:
                # loads have no deps, so their DMAs overlap the G0 matmuls
                whh0 = prec.tile([128, KCH, 2 * 4 * HD], BF16, name="whh0")
                nc.sync.dma_start(
                    out=whh0[:], in_=ins["whh0T"].rearrange("(k p) m -> p k m", p=128)
                )
                whh1 = prec.tile([128, KCH, 2 * 4 * HD], BF16, name="whh1")
                nc.sync.dma_start(
                    out=whh1[:], in_=ins["whh1T"].rearrange("(k p) m -> p k m", p=128)
                )
                wih1 = prec.tile([128, DCH, 2 * 4 * HD], BF16, name="wih1")
                nc.sync.dma_start(
                    out=wih1[:], in_=ins["wih1T"].rearrange("(k p) m -> p k m", p=128)
                )
                G1_sb = prec.tile([128, 2 * MCH, NT], BF16, name="G1_sb")
                b1rep = prec.tile([128, 2 * MCH, GB], BF16, name="b1rep")
                for bq in range(GB):
                    nc.vector.tensor_copy(
                        out=b1rep[:, :, bq : bq + 1],
                        in_=b_sb[:, 2 * MCH : 4 * MCH].rearrange("p (m o) -> p m o", o=1),
                    )
                if DEBUG_OUTS:
                    nc.sync.dma_start(out=dbg["dbg_g"], in_=G_sb[:])
                # fold the DVE- and ACT-written G halves into PE's clock so
                # the recurrence's injects/matmuls carry one sem wait each.
                pe_touch(G_sb[:, 0, 0:1])
                pe_touch(G_sb[:, 0, 512:513])
                with tc.tile_pool(name="r0st", bufs=1, space="PSUM") as spool:
                    recurrence(
                        0, G_sb, h0f, h0b, whh0, stream=(G1_sb, wih1, spool, b1rep)
                    )
                if DEBUG_OUTS:
                    nc.sync.dma_start(out=dbg["dbg_h0f"], in_=h0f[:])
                    nc.sync.dma_start(out=dbg["dbg_h0b"], in_=h0b[:])
                pe_touch(G1_sb[:, 0, 0:1])
                recurrence(1, G1_sb, h1f, h1b, whh1)
        if DEBUG_OUTS:
            nc.sync.dma_start(out=dbg["dbg_h1f"], in_=h1f[:])
            nc.sync.dma_start(out=dbg["dbg_h1b"], in_=h1b[:])

        # ---- emissions: em^T [C, NT] = fc @ concat(h1f, h1b) + fc_b ----
        crf_cm = tc.tile_pool(name="crf", bufs=1)
        crf = crf_cm.__enter__()
        labf_sb = crf.tile([1, NT], F32, name="labf_sb")
        nc.sync.dma_start(out=labf_sb[:], in_=ins["labf"])
        pe_touch(cpack_sb[:, 0:1])
        tileA = crf.tile([C, NT], F32, name="tileA")  # emT, later M1/pd
        tileB = crf.tile([C, NT], F32, name="tileB")  # Q
        tileC = crf.tile([C, NT], F32, name="tileC")  # lab_bc, later gem
        tileD = crf.tile([C, NT], F32, name="tileD")  # OH
        emT = tileA
        with tc.tile_pool(name="emps", bufs=2, space="PSUM") as emps:
            nc.tensor.ldweights(weights=fcT_sb[:, 0, 0:1])
            for nb in range(NT // 512):
                ps = emps.tile([128, 512], F32, tag="emps")
                for kk in range(DCH):
                    src = h1f if kk < KCH else h1b
                    nc.tensor.matmul(
                        out=ps[:C, :],
                        lhsT=fcT_sb[:, kk, :],
                        rhs=src[:, kk % KCH, 512 * nb : 512 * (nb + 1)],
                        start=(kk == 0),
                        stop=(kk == DCH - 1),
                    )
                nc.vector.tensor_scalar_add(
                    out=emT[:, 512 * nb : 512 * (nb + 1)], in0=ps[:C, :], scalar1=fcb_sb[:]
                )
        if DEBUG_OUTS:
            nc.sync.dma_start(out=dbg["dbg_em"], in_=emT[:])

        # ---- CRF ----
        with tc.tile_pool(name="crfw", bufs=4) as cw, tc.tile_pool(
            name="crfps", bufs=1, space="PSUM"
        ) as cps:
            Q = tileB
            nc.scalar.activation(out=Q[:], in_=emT[:], func=AF.Exp)
            dve_touch(Q[0:1, 0:1])
            Qv = Q[:].rearrange("c (b t) -> c b t", b=GB)

            # Bidirectional scan in exp domain, meeting at s = T/2 - 1:
            #   alpha:  v_t = (E^T v_{t-1}) * q_t,        t = 1..s
            #   beta:   b_{t-1} = E (q_t * b_t),          t = T-1..s+1
            #   Z = sum_i v_s[i] * b_s[i]
            # Renorm is LAZY: the 1/s scale from a renorm is folded into
            # that chain's q a few steps in its own future (off the serial
            # chain); ln(s) values are batch-processed at the end.
            LAG = 3
            NREN = 32
            TMID = T // 2          # meet at s = TMID - 1
            s_store = cw.tile([1, GB, NREN], F32, tag="s_store")
            nc.vector.memset(s_store[:], 1.0)
            v_prev = cw.tile([C, GB], F32, tag="v")
            nc.vector.tensor_scalar_mul(out=v_prev[:], in0=Qv[:, :, 0], scalar1=expst_sb[:])
            b_ps = None            # beta state lives in PSUM between steps
            qs_a = {}
            qs_b = {}
            nren_a, nren_b = 0, 0

            def renorm(chain_rhs_sb, qcol, pend, slot):
                sps = cps.tile([1, GB], F32, tag="cps1", bufs=1, name="sps")
                nc.tensor.matmul(
                    out=sps[:], lhsT=onesC1[:], rhs=chain_rhs_sb, start=True, stop=True
                )
                nc.vector.tensor_copy(out=s_store[:, :, slot], in_=sps[:])
                rv = cw.tile([1, GB], F32, tag="rv", name="rv")
                nc.vector.reciprocal(out=rv[:], in_=s_store[:, :, slot])
                rvb = cps.tile([C, GB], F32, tag="rvb", bufs=1, name="rvb")
                nc.tensor.matmul(out=rvb[:], lhsT=ones1C[:], rhs=rv[:], start=True, stop=True)
                qs = cw.tile([C, GB], F32, tag="qs", bufs=4, name="qs")
                nc.vector.tensor_tensor(out=qs[:], in0=rvb[:], in1=Qv[:, :, qcol], op=ALU.mult)
                pend[qcol] = qs

            # --- score-prep as spaced tasks run inside the scan's idle
            # windows (big ops split in half to bound queue-head stalls) ---
            lab_bc = tileC
            OH = tileD
            gem = tileC
            M1 = tileA
            gem_r2 = cw.tile([C, 2], F32, tag="gred2")
            pd_r2 = cw.tile([C, 2], F32, tag="pdr2")
            st_r = cw.tile([C, 1], F32, tag="str")
            en_r = cw.tile([C, 1], F32, tag="enr")
            score_sb = cw.tile([1, 1], F32, tag="scoresb")
            OHv = OH[:].rearrange("c (b t) -> c b t", b=GB)
            pdv = M1[:].rearrange("c (b t) -> c b t", b=GB)

            def tk_lab(nb):
                bps = cps.tile([C, 512], F32, tag="cps512", name="bps")
                nc.tensor.matmul(
                    out=bps[:],
                    lhsT=ones1C[:],
                    rhs=labf_sb[:, 512 * nb : 512 * (nb + 1)],
                    start=True,
                    stop=True,
                )
                nc.vector.tensor_copy(out=lab_bc[:, 512 * nb : 512 * (nb + 1)], in_=bps[:])

            def tk_oh(nb):
                nc.vector.tensor_scalar(
                    out=OH[:, 512 * nb : 512 * (nb + 1)],
                    in0=lab_bc[:, 512 * nb : 512 * (nb + 1)],
                    scalar1=iota_sb[:],
                    scalar2=None,
                    op0=ALU.is_equal,
                )

            def tk_gem(nb):
                nc.vector.tensor_tensor(
                    out=gem[:, 512 * nb : 512 * (nb + 1)],
                    in0=emT[:, 512 * nb : 512 * (nb + 1)],
                    in1=OH[:, 512 * nb : 512 * (nb + 1)],
                    op=ALU.mult,
                )

            def tk_gem_r(nb):
                nc.vector.reduce_sum(
                    out=gem_r2[:, nb : nb + 1],
                    in_=gem[:, 512 * nb : 512 * (nb + 1)],
                    axis=mybir.AxisListType.X,
                )

            def tk_m1(nb):
                lo = 512 * nb
                hi = min(512 * (nb + 1), NT - 1)
                mps = cps.tile([C, 512], F32, tag="cps512", name="mps")
                nc.tensor.matmul(
                    out=mps[:, : hi - lo],
                    lhsT=transT_sb[:],
                    rhs=OH[:, lo + 1 : hi + 1],
                    start=True,
                    stop=True,
                )
                nc.vector.tensor_copy(out=M1[:, lo:hi], in_=mps[:, : hi - lo])

            def tk_m1mult(nb):
                lo = 512 * nb
                hi = min(512 * (nb + 1), NT - 1)
                nc.vector.tensor_tensor(
                    out=M1[:, lo:hi], in0=OH[:, lo:hi], in1=M1[:, lo:hi], op=ALU.mult
                )

            def tk_pd(half):
                nc.vector.reduce_sum(
                    out=pd_r2[:, half : half + 1],
                    in_=pdv[:, 2 * half : 2 * half + 2, 0 : T - 1],
                    axis=mybir.AxisListType.XY,
                )

            def tk_sten():
                st8 = cw.tile([C, GB], F32, tag="st8", name="st8")
                nc.vector.tensor_scalar_mul(out=st8[:], in0=OHv[:, :, 0], scalar1=stv_sb[:])
                nc.vector.reduce_sum(out=st_r[:], in_=st8[:], axis=mybir.AxisListType.X)
                en8 = cw.tile([C, GB], F32, tag="en8", name="en8")
                nc.vector.tensor_scalar_mul(out=en8[:], in0=OHv[:, :, T - 1], scalar1=env_sb[:])
                nc.vector.reduce_sum(out=en_r[:], in_=en8[:], axis=mybir.AxisListType.X)

            def tk_score():
                score_ps = pscr  # reuse the absorber PSUM bank
                parts = (
                    gem_r2[:, 0:1], gem_r2[:, 1:2],
                    pd_r2[:, 0:1], pd_r2[:, 1:2],
                    st_r[:], en_r[:],
                )
                for q, r in enumerate(parts):
                    nc.tensor.matmul(
                        out=score_ps[:1, :1],
                        lhsT=onesC1[:],
                        rhs=r,
                        start=(q == 0),
                        stop=(q == len(parts) - 1),
                        skip_group_check=True,
                    )
                nc.vector.tensor_copy(out=score_sb[:], in_=score_ps[:1, :1])

            tasks = [
                lambda: tk_lab(0), lambda: tk_lab(1),
                lambda: tk_oh(0), lambda: tk_oh(1),
                lambda: tk_gem(0), lambda: tk_gem(1),
                lambda: tk_gem_r(0), lambda: tk_gem_r(1),
                lambda: tk_m1(0), lambda: tk_m1(1),
                lambda: tk_m1mult(0), lambda: tk_m1mult(1),
                lambda: tk_pd(0), lambda: tk_pd(1),
                tk_sten, tk_score,
            ]

            for i in range(TMID):
                if tasks and i % 4 == 1:
                    tasks.pop(0)()
                # --- alpha step t = i+1 (runs for i = 0..TMID-2) ---
                ta = i + 1
                if ta <= TMID - 1:
                    vps = cps.tile([C, GB], F32, tag="vps", bufs=2, name="vps")
                    nc.tensor.matmul(
                        out=vps[:], lhsT=E_sb[:], rhs=v_prev[:], start=True, stop=True
                    )
                    v_new = cw.tile([C, GB], F32, tag="v", name="v_new")
                    qt = qs_a.pop(ta, None)
                    nc.vector.tensor_tensor(
                        out=v_new[:],
                        in0=vps[:],
                        in1=(qt[:] if qt is not None else Qv[:, :, ta]),
                        op=ALU.mult,
                    )
                    v_prev = v_new
                    if ta % RENORM == RENORM - 1 and ta + LAG <= TMID - 1:
                        renorm(v_prev[:], ta + LAG, qs_a, nren_a)
                        nren_a += 1
                # --- beta step t = T-1-i: w = q_t * b_t ; b_{t-1} = E w ---
                tb = T - 1 - i
                w = cw.tile([C, GB], F32, tag="w", name="w")
                qt = qs_b.pop(tb, None)
                qin = qt[:] if qt is not None else Qv[:, :, tb]
                if b_ps is None:
                    nc.vector.tensor_scalar_mul(out=w[:], in0=qin, scalar1=expen_sb[:])
                else:
                    nc.vector.tensor_tensor(out=w[:], in0=qin, in1=b_ps[:], op=ALU.mult)
                b_ps = cps.tile([C, GB], F32, tag="bps", bufs=2, name="b_ps")
                nc.tensor.matmul(out=b_ps[:], lhsT=expTT_sb[:], rhs=w[:], start=True, stop=True)
                j = i + 1  # beta steps completed
                if j % RENORM == 3 and i + 1 + LAG < TMID:
                    renorm(w[:], tb - 1 - LAG, qs_b, 16 + nren_b)
                    nren_b += 1

            # Z = colsum(v_mid * b_mid)
            zv = cw.tile([C, GB], F32, tag="zv")
            nc.vector.tensor_tensor(out=zv[:], in0=v_prev[:], in1=b_ps[:], op=ALU.mult)
            zps = cps.tile([1, GB], F32, tag="cps1", bufs=1)
            nc.tensor.matmul(out=zps[:], lhsT=onesC1[:], rhs=zv[:], start=True, stop=True)
            lnz = cw.tile([1, GB], F32, tag="lnz")
            nc.scalar.activation(out=lnz[:], in_=zps[:], func=AF.Ln)
            lns_all = cw.tile([1, GB, NREN], F32, tag="lns_all")
            nc.scalar.activation(out=lns_all[:], in_=s_store[:], func=AF.Ln)
            off = cw.tile([1, GB], F32, tag="off")
            nc.vector.reduce_sum(out=off[:], in_=lns_all[:], axis=mybir.AxisListType.X)
            logz = cw.tile([1, GB], F32, tag="logz")
            nc.vector.tensor_tensor(out=logz[:], in0=lnz[:], in1=off[:], op=ALU.add)
            lz_tot = cw.tile([1, 1], F32, tag="lztot")
            nc.vector.reduce_sum(out=lz_tot[:], in_=logz[:], axis=mybir.AxisListType.X)
            loss_sb = cw.tile([1, 1], F32, tag="loss_sb")
            nc.vector.tensor_tensor(out=loss_sb[:], in0=lz_tot[:], in1=score_sb[:], op=ALU.subtract)
            nc.sync.dma_start(out=loss_out, in_=loss_sb[:])
            if DEBUG_OUTS:
                dsc = cw.tile([1, 2], F32, tag="dsc")
                nc.vector.tensor_copy(out=dsc[:, 0:1], in_=lz_tot[:])
                nc.vector.tensor_copy(out=dsc[:, 1:2], in_=score_sb[:])
                nc.sync.dma_start(out=dbg["dbg_sc"], in_=dsc[:])
        crf_cm.__exit__(None, None, None)

    est.close()


# ---------------------------------------------------------------------------
# host side
# ---------------------------------------------------------------------------

def make_in_maps(inputs):
    ids = np.asarray(inputs["input_ids"]).astype(np.int64)
    labels = np.asarray(inputs["labels"]).astype(np.int64)
    word_emb = _f32(inputs["word_emb"])
    pos_emb = _f32(inputs["pos_emb"])
    type_emb = _f32(inputs["type_emb"])
    ln_g = _f32(inputs["ln_g"])
    ln_b = _f32(inputs["ln_b"])
    w_ih = _f32(inputs["w_ih"])
    w_hh = _f32(inputs["w_hh"])
    b_ih = _f32(inputs["b_ih"])
    b_hh = _f32(inputs["b_hh"])
    fc_w = _f32(inputs["fc_w"])
    fc_b = _f32(inputs["fc_b"])
    crf_start = _f32(inputs["crf_start"])
    crf_end = _f32(inputs["crf_end"])
    crf_trans = _f32(inputs["crf_trans"])

    posty = np.ascontiguousarray(pos_emb[:T] + type_emb[0][None, :])

    def sig_trick(w, b):
        # fold tanh(x) = 2*sigmoid(2x)-1: scale g-gate rows (2HD:3HD) by 2
        w = w.copy()
        b = b.copy()
        w[2 * HD : 3 * HD] *= 2.0
        b[2 * HD : 3 * HD] *= 2.0
        return w, b

    # per-layer packed weights, both directions: cols [fwd 4HD | bwd 4HD]
    wihT = []
    whhT = []
    biases = []  # [l][d] -> (1536,)
    for l in range(L):
        wl = []
        hl = []
        bl = []
        for d in range(2):
            w = w_ih[l, d]
            bias = b_ih[l, d] + b_hh[l, d]
            if l == 0:
                bias = bias + w @ ln_b
                w = w * ln_g[None, :]
            u = w_hh[l, d]
            w, bias = sig_trick(w, bias)
            u, _ = sig_trick(u, np.zeros(4 * HD, np.float32))
            wl.append(w.T)   # [in_dim, 1536]
            hl.append(u.T)   # [HD, 1536]
            bl.append(bias)
        wihT.append(np.concatenate(wl, axis=1))   # [in_dim, 3072]
        whhT.append(np.concatenate(hl, axis=1))   # [HD, 3072]
        biases.append(bl)

    # b01 [128, 48]: col l*24 + d*12 + m holds bias[l][d][128m:128(m+1)]
    bcols = []
    for l in range(L):
        for d in range(2):
            bcols.append(biases[l][d].reshape(MCH, 128).T)
    b01 = np.ascontiguousarray(np.concatenate(bcols, axis=1))

    cpack = np.zeros((C, 48), np.float32)
    cpack[:, 0:C] = np.exp(crf_trans)
    cpack[:, C : 2 * C] = crf_trans.T
    cpack[:, 28] = np.exp(crf_start)
    cpack[:, 29] = np.exp(crf_end)
    cpack[:, 30] = crf_start
    cpack[:, 31] = crf_end
    cpack[:, 32] = np.arange(C, dtype=np.float32)
    cpack[:, 33] = fc_b
    cpack[:, 34:48] = np.exp(crf_trans).T

    shared = dict(
        word_emb=word_emb,
        posty=posty,
        wih0T=_bf(wihT[0]),
        wih1T=_bf(wihT[1]),
        whh0T=_bf(whhT[0]),
        whh1T=_bf(whhT[1]),
        b01=b01,
        fcT=_bf(fc_w.T),
        cpack=cpack,
    )

    in_maps = []
    for core in range(NCORES):
        sl = slice(GB * core, GB * (core + 1))
        in_maps.append(
            dict(
                ids32=np.ascontiguousarray(ids[sl].reshape(NT, 1).astype(np.int32)),
                labf=np.ascontiguousarray(labels[sl].reshape(1, NT).astype(np.float32)),
                **shared,
            )
        )
    return in_maps


_PROGRAM = None
_COST_MODEL_NS = None


def _get_program():
    global _PROGRAM, _COST_MODEL_NS
    if _PROGRAM is None:
        _PROGRAM = build_program()
        try:
            from concourse.timeline_sim import TimelineSim

            _COST_MODEL_NS = int(TimelineSim(_PROGRAM, trace=False, no_exec=True).simulate())
        except Exception:
            _COST_MODEL_NS = None
    return _PROGRAM


def run(inputs, trace=False):
    nc = _get_program()
    in_maps = make_in_maps(inputs)
    res = run_bass_kernel_spmd(nc, in_maps, core_ids=list(range(NCORES)), trace=trace)
    total = np.float64(0.0)
    for c in range(NCORES):
        total += np.float64(res.results[c]["loss"][0, 0])
    return np.asarray(total, dtype=np.float32), res


def kernel(**inputs):
    out, _ = run(inputs, trace=False)
    return out


# revision 62
# speedup vs baseline: 1.0039x; 1.0003x over previous
"""BiLSTM-CRF sequence-tagging loss on 8 Trainium2 NeuronCores.

Sharding: pure data-parallel — core c owns sequences [4c, 4c+4) and runs
BOTH LSTM directions locally (no collectives at all).  The backward
direction writes its h-stream time-reversed directly (free AP offset), so
layer-1/emission inputs are plain [h_fwd | h_bwd] concats.

Per-step recurrence structure (per direction, chains interleaved so the
two directions hide each other's latency):
  PE   : G+bias injected into PSUM via identity matmul (prefetched one
         step ahead into the alternate bank) + 36 Whh matmuls.
  ACT  : one Sigmoid over all 12 gate chunks — the g-gate uses
         tanh(x) = 2*sigmoid(2x) - 1 with the 2x folded into the weights.
  DVE  : u = (sg - 0.5) * si ; t1 = sf * c_prev ; c = 2u + t1 (fused
         scalar_tensor_tensor ops).
  ACT  : tc = tanh(c)
  DVE  : h = so * tc  (written straight into the bf16 h stream).
"""

import os
import sys

import numpy as np

for _p in ("/opt/trn_rl_repo", "/root/.axon_site/_ro/trn_rl_repo"):
    if os.path.isdir(_p) and _p not in sys.path:
        sys.path.insert(0, _p)

import ml_dtypes  # noqa: E402

import concourse.bass as bass  # noqa: E402
import concourse.bacc as bacc  # noqa: E402
import concourse.tile as tile  # noqa: E402
from concourse import mybir  # noqa: E402
from concourse.bass import IndirectOffsetOnAxis  # noqa: E402
from concourse.bass_utils import run_bass_kernel_spmd  # noqa: E402
from concourse.masks import make_identity  # noqa: E402

F32 = mybir.dt.float32
BF16 = mybir.dt.bfloat16
I32 = mybir.dt.int32
AF = mybir.ActivationFunctionType
ALU = mybir.AluOpType

# problem shapes (hardcoded per contract)
B, T, V, D, C, HD = 32, 256, 30522, 768, 14, 384
L = 2
NCORES = 8
GB = 4             # sequences per core
NT = GB * T        # tokens per core = 1024
NTILE = NT // 128  # 8
MCH = 12           # gate chunks of 128 per direction (4*HD/128)
KCH = 3            # hidden chunks per direction (HD/128)
DCH = 6            # input-dim chunks (D/128)
LN_EPS = 1e-12
RENORM = 8

DEBUG_OUTS = False


def _bf(x):
    return np.ascontiguousarray(np.asarray(x, dtype=np.float32)).astype(ml_dtypes.bfloat16)


def _f32(x):
    return np.ascontiguousarray(np.asarray(x, dtype=np.float32))


# ---------------------------------------------------------------------------
# device program
# ---------------------------------------------------------------------------

def build_program():
    nc = bacc.Bacc("TRN2", target_bir_lowering=False, debug=False, num_devices=NCORES)

    def din(name, shape, dt):
        return nc.dram_tensor(name, shape, dt, kind="ExternalInput").ap()

    ins = dict(
        ids32=din("ids32", [NT, 1], I32),
        labf=din("labf", [1, NT], F32),
        word_emb=din("word_emb", [V, D], F32),
        posty=din("posty", [T, D], F32),
        wih0T=din("wih0T", [D, 2 * 4 * HD], BF16),
        wih1T=din("wih1T", [D, 2 * 4 * HD], BF16),
        whh0T=din("whh0T", [HD, 2 * 4 * HD], BF16),
        whh1T=din("whh1T", [HD, 2 * 4 * HD], BF16),
        b01=din("b01", [128, 4 * MCH], F32),
        fcT=din("fcT", [D, C], BF16),
        cpack=din("cpack", [C, 48], F32),
    )

    loss_out = nc.dram_tensor("loss", [1, 1], F32, kind="ExternalOutput").ap()
    dbg = {}
    if DEBUG_OUTS:
        dbg["dbg_xt"] = nc.dram_tensor("dbg_xt", [128, DCH, NT], BF16, kind="ExternalOutput").ap()
        dbg["dbg_g"] = nc.dram_tensor("dbg_g", [128, 2 * MCH, NT], BF16, kind="ExternalOutput").ap()
        dbg["dbg_h0f"] = nc.dram_tensor("dbg_h0f", [128, KCH, NT], BF16, kind="ExternalOutput").ap()
        dbg["dbg_h0b"] = nc.dram_tensor("dbg_h0b", [128, KCH, NT], BF16, kind="ExternalOutput").ap()
        dbg["dbg_h1f"] = nc.dram_tensor("dbg_h1f", [128, KCH, NT], BF16, kind="ExternalOutput").ap()
        dbg["dbg_h1b"] = nc.dram_tensor("dbg_h1b", [128, KCH, NT], BF16, kind="ExternalOutput").ap()
        dbg["dbg_em"] = nc.dram_tensor("dbg_em", [C, NT], F32, kind="ExternalOutput").ap()
        dbg["dbg_sc"] = nc.dram_tensor("dbg_sc", [1, 2], F32, kind="ExternalOutput").ap()

    with tile.TileContext(nc) as tc:
        _build_body(tc, ins, loss_out, dbg)

    nc.compile()
    return nc


def _build_body(tc, ins, loss_out, dbg):
    nc = tc.nc
    from contextlib import ExitStack

    est = ExitStack()
    pers = est.enter_context(tc.tile_pool(name="pers", bufs=1))

    # scratch + absorbers: only ONE sem wait per instruction is allowed, so
    # junctions of two producers get a tiny absorber op that folds one
    # producer into the consuming engine's clock first.
    scr_dve = pers.tile([1, 4], F32, name="scr_dve")
    scr_act = pers.tile([1, 4], F32, name="scr_act")
    pabs = est.enter_context(tc.tile_pool(name="pabs", bufs=1, space="PSUM"))
    pscr = pabs.tile([1, 8], F32, name="pscr")

    scr_gp = pers.tile([1, 4], F32, name="scr_gp")

    def dve_touch(ap):
        nc.vector.tensor_copy(out=scr_dve[:, 0:1], in_=ap)

    def gp_touch(ap):
        nc.gpsimd.tensor_copy(out=scr_gp[:, 0:1], in_=ap)

    def act_touch(ap):
        nc.scalar.activation(out=scr_act[:, 0:1], in_=ap, func=AF.Copy)

    def pe_touch(ap_col):
        nc.tensor.matmul(out=pscr[:1, :1], lhsT=ap_col, rhs=ap_col, start=True, stop=True)

    b_sb = pers.tile([128, 4 * MCH], F32, name="b_sb")
    nc.sync.dma_start(out=b_sb[:], in_=ins["b01"])
    dve_touch(b_sb[0:1, 0:1])
    act_touch(b_sb[0:1, 0:1])

    fcT_sb = pers.tile([128, DCH, C], BF16, name="fcT")
    nc.sync.dma_start(out=fcT_sb[:], in_=ins["fcT"].rearrange("(k p) m -> p k m", p=128))

    cpack_sb = pers.tile([C, 48], F32, name="cpack_sb")
    nc.sync.dma_start(out=cpack_sb[:], in_=ins["cpack"])
    dve_touch(cpack_sb[0:1, 0:1])
    E_sb = cpack_sb[:, 0:C]
    transT_sb = cpack_sb[:, C : 2 * C]
    expst_sb = cpack_sb[:, 28:29]
    expen_sb = cpack_sb[:, 29:30]
    stv_sb = cpack_sb[:, 30:31]
    env_sb = cpack_sb[:, 31:32]
    iota_sb = cpack_sb[:, 32:33]
    fcb_sb = cpack_sb[:, 33:34]
    expTT_sb = cpack_sb[:, 34:48]

    ids_sb = pers.tile([128, NTILE], I32, name="ids_sb")
    nc.sync.dma_start(out=ids_sb[:], in_=ins["ids32"].rearrange("(k p) o -> p (k o)", p=128))

    ident = pers.tile([128, 128], F32, name="ident")
    make_identity(nc, ident[:])
    pe_touch(ident[:, 0:1])
    identb = pers.tile([128, 128], BF16, name="identb")
    make_identity(nc, identb[:])
    pe_touch(identb[:, 0:1])
    eps_sb = pers.tile([128, 1], F32, name="eps_sb")
    nc.vector.memset(eps_sb[:], LN_EPS)
    ones1C = pers.tile([1, C], F32, name="ones1C")
    nc.vector.memset(ones1C[:], 1.0)
    onesC1 = pers.tile([C, 1], F32, name="onesC1")
    nc.vector.memset(onesC1[:], 1.0)

    # ---- helpers ----
    def s1_embed(xT_sb, ks, posty_sb):
        with tc.tile_pool(name="s1", bufs=3) as s1, tc.tile_pool(
            name="s1ps", bufs=4, space="PSUM"
        ) as s1ps:
            for k in ks:
                emb = s1.tile([128, D], F32, tag="emb")
                nc.gpsimd.indirect_dma_start(
                    out=emb[:],
                    out_offset=None,
                    in_=ins["word_emb"],
                    in_offset=IndirectOffsetOnAxis(ap=ids_sb[:, k : k + 1], axis=0),
                )
                nc.vector.tensor_add(out=emb[:], in0=emb[:], in1=posty_sb[:, k % 2, :])
                stats = s1.tile([128, 3, 6], F32, tag="stats")
                embv = emb[:].rearrange("p (s q) -> p s q", s=3)
                for sg_ in range(3):
                    nc.vector.bn_stats(out=stats[:, sg_, :], in_=embv[:, sg_, :])
                mv = s1.tile([128, 2], F32, tag="mv")
                nc.vector.bn_aggr(out=mv[:], in_=stats[:])
                std = s1.tile([128, 1], F32, tag="std")
                nc.scalar.activation(out=std[:], in_=mv[:, 1:2], func=AF.Sqrt, bias=eps_sb[:])
                rstd = s1.tile([128, 1], F32, tag="rstd")
                nc.vector.reciprocal(out=rstd[:], in_=std[:])
                xln = s1.tile([128, D], F32, tag="xln")
                nc.vector.tensor_scalar(
                    out=xln[:],
                    in0=emb[:],
                    scalar1=mv[:, 0:1],
                    scalar2=rstd[:],
                    op0=ALU.subtract,
                    op1=ALU.mult,
                )
                kl = k - ks[0]
                for j in range(DCH):
                    tp = s1ps.tile([128, 128], F32, tag="tp")
                    nc.tensor.transpose(
                        out=tp[:], in_=xln[:, 128 * j : 128 * (j + 1)], identity=ident[:]
                    )
                    if j % 2 == 0:
                        nc.vector.tensor_copy(
                            out=xT_sb[:, j, 128 * kl : 128 * (kl + 1)], in_=tp[:]
                        )
                    else:
                        nc.scalar.activation(
                            out=xT_sb[:, j, 128 * kl : 128 * (kl + 1)], in_=tp[:], func=AF.Copy
                        )

    def g_matmul(l, G_sb, rhs_of, wih, nbs=None, tag="", ncol=512):
        """G = x @ Wih^T + bias for both dirs, written into G_sb (bf16)."""
        with tc.tile_pool(name=f"g{l}ps{tag}", bufs=6, space="PSUM") as gps:
            nc.tensor.ldweights(weights=wih[:, 0, 0:1])
            for m in range(2 * MCH):
                for nb in (range(NT // ncol) if nbs is None else nbs):
                    ps = gps.tile([128, ncol], F32, tag="gps", name="gps")
                    for kk in range(DCH):
                        nc.tensor.matmul(
                            out=ps[:],
                            lhsT=wih[:, kk, 128 * m : 128 * (m + 1)],
                            rhs=rhs_of(kk, nb),
                            start=(kk == 0),
                            stop=(kk == DCH - 1),
                        )
                    bcol = b_sb[:, l * 2 * MCH + m : l * 2 * MCH + m + 1]
                    if (m + nb) % 2 == 0:
                        nc.vector.tensor_scalar_add(
                            out=G_sb[:, m, ncol * nb : ncol * (nb + 1)],
                            in0=ps[:],
                            scalar1=bcol,
                        )
                    else:
                        nc.scalar.activation(
                            out=G_sb[:, m, ncol * nb : ncol * (nb + 1)],
                            in_=ps[:],
                            func=AF.Identity,
                            bias=bcol,
                        )

    def recurrence(l, G_sb, hf, hb, whh, bias_rep=None, stream=None):
        """Both-direction LSTM over local time, staggered chains.

        Direction d reads G chunks [12d:12d+12] at time col (fwd: t, bwd:
        T-1-t) and writes its h at the same time col -> both h streams end
        up time-ordered.

        bias_rep: [128, 24, GB] bias tile injected into PSUM with G (used
        when G_sb itself carries no bias).
        stream: (G1_sb, wih1, spool) — accumulate the NEXT layer's G into
        G1_sb column-by-column as h values appear (fills PE/DVE idle)."""
        Gv = G_sb[:].rearrange("p m (b t) -> p m b t", b=GB)
        hv = [h[:].rearrange("p c (b t) -> p c b t", b=GB) for h in (hf, hb)]
        if stream is not None:
            G1_sb, wih1s, spool, bias1s = stream
            G1v = G1_sb[:].rearrange("p m (b t) -> p m b t", b=GB)
            nc.tensor.ldweights(weights=wih1s[:, 0, 0:1])

        with tc.tile_pool(name=f"r{l}", bufs=4) as rp, tc.tile_pool(
            name=f"r{l}c", bufs=2
        ) as rcp, tc.tile_pool(
            name=f"r{l}psA", bufs=3, space="PSUM"
        ) as rpsA, tc.tile_pool(
            name=f"r{l}psB", bufs=2, space="PSUM"
        ) as rpsB:
            rps = [rpsA, rpsB]
            ps_cur = [None, None]   # PSUM tile holding this step's preacts
            ps_nxt = [None, None]
            c_prev = [None, None]
            nc.tensor.ldweights(weights=whh[:, 0, 0:1])

            stream_pending = []
            sps_done = []

            def tcol(d, t):
                return t if d == 0 else T - 1 - t

            def emit_stream_mm(d, col):
                sp = spool.tile([128, 2 * MCH, GB], F32, tag=f"sp{d}", name="sp")
                first = (col <= T // 2 - 1) if d == 0 else (col >= T // 2)
                # open the accumulation with the bias (first touch) or the
                # column's current partial sum (second touch), so the final
                # move is a plain copy on either engine.
                nc.tensor.matmul(
                    out=sp[:],
                    lhsT=identb[:],
                    rhs=(bias1s[:] if first else G1v[:, :, :, col]),
                    start=True,
                    stop=False,
                    skip_group_check=True,
                )
                for m in range(2 * MCH):
                    for kj in range(KCH):
                        nc.tensor.matmul(
                            out=sp[:, m, :],
                            lhsT=wih1s[:, kj + KCH * d, 128 * m : 128 * (m + 1)],
                            rhs=hv[d][:, kj, :, col],
                            start=False,
                            stop=(kj == KCH - 1),
                            skip_group_check=True,
                        )
                return sp

            def emit_stream_move(d, col, sp):
                # plain copy, alternated across ACT/DVE by direction so each
                # engine absorbs one move per step in its idle window.
                if d == 0:
                    nc.scalar.activation(out=G1v[:, :, :, col], in_=sp[:], func=AF.Copy)
                else:
                    nc.vector.tensor_copy(out=G1v[:, :, :, col], in_=sp[:])

            def emit_inject(d, tt, close):
                psn = rps[d].tile([128, MCH, GB], F32, tag=f"ps{d}", name=f"psd")
                nc.tensor.matmul(
                    out=psn[:],
                    lhsT=identb[:],
                    rhs=Gv[:, MCH * d : MCH * (d + 1), :, tcol(d, tt)],
                    start=True,
                    stop=(close and bias_rep is None),
                    skip_group_check=True,
                )
                if bias_rep is not None:
                    nc.tensor.matmul(
                        out=psn[:],
                        lhsT=ident[:],
                        rhs=bias_rep[:, MCH * d : MCH * (d + 1), :],
                        start=False,
                        stop=close,
                        skip_group_check=True,
                    )
                return psn

            if bias_rep is not None:
                # t=0 preacts also come via PSUM (G_sb carries no bias)
                for d in (0, 1):
                    ps_cur[d] = emit_inject(d, 0, close=True)

            for t in range(T):
                sg = [None, None]
                # --- flush stream moves whose PSUM inputs are ready; at the
                # top of the cycle both ACT and DVE are idle waiting on the
                # chain, so these are free ---
                if stream is not None:
                    for d, col, sp in sps_done:
                        emit_stream_move(d, col, sp)
                    sps_done = []
                # --- PE: Whh matmuls into the prefetched PSUM bank ---
                if t > 0:
                    for d in (0, 1):
                        ps = ps_cur[d]
                        hprev = hv[d][:, :, :, tcol(d, t - 1)]
                        for kk in range(KCH):
                            for m in range(MCH):
                                nc.tensor.matmul(
                                    out=ps[:, m, :],
                                    lhsT=whh[
                                        :, kk, 1536 * d + 128 * m : 1536 * d + 128 * (m + 1)
                                    ],
                                    rhs=hprev[:, kk, :],
                                    start=False,
                                    stop=(kk == KCH - 1 and m == MCH - 1),
                                    skip_group_check=True,
                                )
                # --- ACT: sigmoid over all 12 gate chunks (both dirs) ---
                for d in (0, 1):
                    sg[d] = rp.tile([128, MCH, GB], F32, tag=f"sg{d}", name=f"sg{d}")
                    if t == 0 and bias_rep is None:
                        src = Gv[:, MCH * d : MCH * (d + 1), :, tcol(d, 0)]
                    else:
                        src = ps_cur[d][:]
                    nc.scalar.activation(out=sg[d][:], in_=src, func=AF.Sigmoid)
                # --- PE: prefetch next step's G+bias into the other bank ---
                if t + 1 < T:
                    for d in (0, 1):
                        ps_nxt[d] = emit_inject(d, t + 1, close=False)
                # --- stream PREVIOUS step's next-layer G column (its h sem
                # is long satisfied, so this runs in PE's idle window after
                # the critical whh burst rather than delaying it) ---
                if stream is not None:
                    for d, col in stream_pending:
                        sps_done.append((d, col, emit_stream_mm(d, col)))
                    stream_pending = []
                # --- DVE: c update ---
                c_new = [None, None]
                for d in (0, 1):
                    u = rp.tile([128, KCH, GB], F32, tag=f"u{d}", name=f"ud")
                    nc.vector.scalar_tensor_tensor(
                        out=u[:],
                        in0=sg[d][:, 6:9, :],
                        scalar=0.5,
                        in1=sg[d][:, 0:3, :],
                        op0=ALU.subtract,
                        op1=ALU.mult,
                    )
                    c_new[d] = rcp.tile([128, KCH, GB], F32, tag=f"c{d}", name=f"cd")
                    if t == 0:
                        nc.vector.tensor_scalar_mul(out=c_new[d][:], in0=u[:], scalar1=2.0)
                    else:
                        t1 = rp.tile([128, KCH, GB], F32, tag=f"t1{d}", name=f"t1d")
                        nc.vector.tensor_tensor(
                            out=t1[:], in0=sg[d][:, 3:6, :], in1=c_prev[d][:], op=ALU.mult
                        )
                        nc.vector.scalar_tensor_tensor(
                            out=c_new[d][:],
                            in0=u[:],
                            scalar=2.0,
                            in1=t1[:],
                            op0=ALU.mult,
                            op1=ALU.add,
                        )
                # --- ACT: tanh(c) ---
                tcs = [None, None]
                for d in (0, 1):
                    tcs[d] = rp.tile([128, KCH, GB], F32, tag=f"tc{d}", name=f"tcd")
                    nc.scalar.activation(out=tcs[d][:], in_=c_new[d][:], func=AF.Tanh)
                # --- DVE: h = sigmoid(o) * tanh(c) ---
                for d in (0, 1):
                    nc.vector.tensor_tensor(
                        out=hv[d][:, :, :, tcol(d, t)],
                        in0=sg[d][:, 9:12, :],
                        in1=tcs[d][:],
                        op=ALU.mult,
                    )
                    c_prev[d] = c_new[d]
                    ps_cur[d] = ps_nxt[d]
                if stream is not None:
                    for d in (0, 1):
                        stream_pending.append((d, tcol(d, t)))
            if stream is not None:
                for d, col, sp in sps_done:
                    emit_stream_move(d, col, sp)
                for d, col in stream_pending:
                    emit_stream_move(d, col, emit_stream_mm(d, col))

    # ---- layer pipeline with scoped lifetimes (strict LIFO pools) ----
    with tc.tile_pool(name="phh", bufs=1) as phh:
        h0f = phh.tile([128, KCH, NT], BF16, name="h0f")
        h0b = phh.tile([128, KCH, NT], BF16, name="h0b")
        h1f = phh.tile([128, KCH, NT], BF16, name="h1f")
        h1b = phh.tile([128, KCH, NT], BF16, name="h1b")
        with tc.tile_pool(name="pg", bufs=1) as pgp:
            G_sb = pgp.tile([128, 2 * MCH, NT], BF16, name="G_sb")
            with tc.tile_pool(name="pw0", bufs=1) as pw0:
                wih0 = pw0.tile([128, DCH, 2 * 4 * HD], BF16, name="wih0")
                nc.sync.dma_start(
                    out=wih0[:], in_=ins["wih0T"].rearrange("(k p) m -> p k m", p=128)
                )
                with tc.tile_pool(name="px", bufs=1) as px:
                    posty_sb = px.tile([128, 2, D], F32, name="posty_sb")
                    nc.sync.dma_start(
                        out=posty_sb[:],
                        in_=ins["posty"].rearrange("(a p) d -> p a d", p=128),
                    )
                    dve_touch(posty_sb[0:1, 0, 0:1])
                    xT_a = px.tile([128, DCH, 512], BF16, name="xT_a")
                    xT_b = px.tile([128, DCH, 512], BF16, name="xT_b")
                    s1_embed(xT_a, range(0, 4), posty_sb)
                    g_matmul(
                        0, G_sb, lambda kk, nb: xT_a[:, kk, :], wih0, nbs=[0], tag="a"
                    )
                    s1_embed(xT_b, range(4, NTILE), posty_sb)
                    g_matmul(
                        0, G_sb, lambda kk, nb: xT_b[:, kk, :], wih0, nbs=[1], tag="b"
                    )
            with tc.tile_pool(name="prec", bufs=1) as prec:
                # loads have no deps, so their DMAs overlap the G0 matmuls
                whh0 = prec.tile([128, KCH, 2 * 4 * HD], BF16, name="whh0")
                nc.sync.dma_start(
                    out=whh0[:], in_=ins["whh0T"].rearrange("(k p) m -> p k m", p=128)
                )
                whh1 = prec.tile([128, KCH, 2 * 4 * HD], BF16, name="whh1")
                nc.sync.dma_start(
                    out=whh1[:], in_=ins["whh1T"].rearrange("(k p) m -> p k m", p=128)
                )
                wih1 = prec.tile([128, DCH, 2 * 4 * HD], BF16, name="wih1")
                nc.sync.dma_start(
                    out=wih1[:], in_=ins["wih1T"].rearrange("(k p) m -> p k m", p=128)
                )
                G1_sb = prec.tile([128, 2 * MCH, NT], BF16, name="G1_sb")
                b1rep = prec.tile([128, 2 * MCH, GB], BF16, name="b1rep")
                for bq in range(GB):
                    nc.vector.tensor_copy(
                        out=b1rep[:, :, bq : bq + 1],
                        in_=b_sb[:, 2 * MCH : 4 * MCH].rearrange("p (m o) -> p m o", o=1),
                    )
                if DEBUG_OUTS:
                    nc.sync.dma_start(out=dbg["dbg_g"], in_=G_sb[:])
                # fold the DVE- and ACT-written G halves into PE's clock so
                # the recurrence's injects/matmuls carry one sem wait each.
                pe_touch(G_sb[:, 0, 0:1])
                pe_touch(G_sb[:, 0, 512:513])
                with tc.tile_pool(name="r0st", bufs=1, space="PSUM") as spool:
                    recurrence(
                        0, G_sb, h0f, h0b, whh0, stream=(G1_sb, wih1, spool, b1rep)
                    )
                if DEBUG_OUTS:
                    nc.sync.dma_start(out=dbg["dbg_h0f"], in_=h0f[:])
                    nc.sync.dma_start(out=dbg["dbg_h0b"], in_=h0b[:])
                pe_touch(G1_sb[:, 0, 0:1])
                recurrence(1, G1_sb, h1f, h1b, whh1)
        if DEBUG_OUTS:
            nc.sync.dma_start(out=dbg["dbg_h1f"], in_=h1f[:])
            nc.sync.dma_start(out=dbg["dbg_h1b"], in_=h1b[:])

        # ---- emissions: em^T [C, NT] = fc @ concat(h1f, h1b) + fc_b ----
        crf_cm = tc.tile_pool(name="crf", bufs=1)
        crf = crf_cm.__enter__()
        labf_sb = crf.tile([1, NT], F32, name="labf_sb")
        nc.sync.dma_start(out=labf_sb[:], in_=ins["labf"])
        pe_touch(cpack_sb[:, 0:1])
        tileA = crf.tile([C, NT], F32, name="tileA")  # emT, later M1/pd
        tileB = crf.tile([C, NT], F32, name="tileB")  # Q
        tileC = crf.tile([C, NT], F32, name="tileC")  # lab_bc, later gem
        tileD = crf.tile([C, NT], F32, name="tileD")  # OH
        emT = tileA
        with tc.tile_pool(name="emps", bufs=4, space="PSUM") as emps:
            nc.tensor.ldweights(weights=fcT_sb[:, 0, 0:1])
            for nb in range(NT // 512):
                ps = emps.tile([128, 512], F32, tag="emps")
                for kk in range(DCH):
                    src = h1f if kk < KCH else h1b
                    nc.tensor.matmul(
                        out=ps[:C, :],
                        lhsT=fcT_sb[:, kk, :],
                        rhs=src[:, kk % KCH, 512 * nb : 512 * (nb + 1)],
                        start=(kk == 0),
                        stop=(kk == DCH - 1),
                    )
                nc.vector.tensor_scalar_add(
                    out=emT[:, 512 * nb : 512 * (nb + 1)], in0=ps[:C, :], scalar1=fcb_sb[:]
                )
        if DEBUG_OUTS:
            nc.sync.dma_start(out=dbg["dbg_em"], in_=emT[:])

        # ---- CRF ----
        with tc.tile_pool(name="crfw", bufs=4) as cw, tc.tile_pool(
            name="crfps", bufs=1, space="PSUM"
        ) as cps:
            Q = tileB
            nc.scalar.activation(out=Q[:], in_=emT[:], func=AF.Exp)
            dve_touch(Q[0:1, 0:1])
            Qv = Q[:].rearrange("c (b t) -> c b t", b=GB)

            # Bidirectional scan in exp domain, meeting at s = T/2 - 1:
            #   alpha:  v_t = (E^T v_{t-1}) * q_t,        t = 1..s
            #   beta:   b_{t-1} = E (q_t * b_t),          t = T-1..s+1
            #   Z = sum_i v_s[i] * b_s[i]
            # Renorm is LAZY: the 1/s scale from a renorm is folded into
            # that chain's q a few steps in its own future (off the serial
            # chain); ln(s) values are batch-processed at the end.
            LAG = 3
            NREN = 32
            TMID = T // 2          # meet at s = TMID - 1
            s_store = cw.tile([1, GB, NREN], F32, tag="s_store")
            nc.vector.memset(s_store[:], 1.0)
            v_prev = cw.tile([C, GB], F32, tag="v")
            nc.vector.tensor_scalar_mul(out=v_prev[:], in0=Qv[:, :, 0], scalar1=expst_sb[:])
            b_ps = None            # beta state lives in PSUM between steps
            qs_a = {}
            qs_b = {}
            nren_a, nren_b = 0, 0

            def renorm(chain_rhs_sb, qcol, pend, slot):
                sps = cps.tile([1, GB], F32, tag="cps1", bufs=1, name="sps")
                nc.tensor.matmul(
                    out=sps[:], lhsT=onesC1[:], rhs=chain_rhs_sb, start=True, stop=True
                )
                nc.vector.tensor_copy(out=s_store[:, :, slot], in_=sps[:])
                rv = cw.tile([1, GB], F32, tag="rv", name="rv")
                nc.vector.reciprocal(out=rv[:], in_=s_store[:, :, slot])
                rvb = cps.tile([C, GB], F32, tag="rvb", bufs=1, name="rvb")
                nc.tensor.matmul(out=rvb[:], lhsT=ones1C[:], rhs=rv[:], start=True, stop=True)
                qs = cw.tile([C, GB], F32, tag="qs", bufs=4, name="qs")
                nc.vector.tensor_tensor(out=qs[:], in0=rvb[:], in1=Qv[:, :, qcol], op=ALU.mult)
                pend[qcol] = qs

            # --- score-prep as spaced tasks run inside the scan's idle
            # windows (big ops split in half to bound queue-head stalls) ---
            lab_bc = tileC
            OH = tileD
            gem = tileC
            M1 = tileA
            gem_r2 = cw.tile([C, 2], F32, tag="gred2")
            pd_r2 = cw.tile([C, 2], F32, tag="pdr2")
            st_r = cw.tile([C, 1], F32, tag="str")
            en_r = cw.tile([C, 1], F32, tag="enr")
            score_sb = cw.tile([1, 1], F32, tag="scoresb")
            OHv = OH[:].rearrange("c (b t) -> c b t", b=GB)
            pdv = M1[:].rearrange("c (b t) -> c b t", b=GB)

            def tk_lab(nb):
                bps = cps.tile([C, 512], F32, tag="cps512", name="bps")
                nc.tensor.matmul(
                    out=bps[:],
                    lhsT=ones1C[:],
                    rhs=labf_sb[:, 512 * nb : 512 * (nb + 1)],
                    start=True,
                    stop=True,
                )
                nc.vector.tensor_copy(out=lab_bc[:, 512 * nb : 512 * (nb + 1)], in_=bps[:])

            def tk_oh(nb):
                nc.vector.tensor_scalar(
                    out=OH[:, 512 * nb : 512 * (nb + 1)],
                    in0=lab_bc[:, 512 * nb : 512 * (nb + 1)],
                    scalar1=iota_sb[:],
                    scalar2=None,
                    op0=ALU.is_equal,
                )

            def tk_gem(nb):
                nc.vector.tensor_tensor(
                    out=gem[:, 512 * nb : 512 * (nb + 1)],
                    in0=emT[:, 512 * nb : 512 * (nb + 1)],
                    in1=OH[:, 512 * nb : 512 * (nb + 1)],
                    op=ALU.mult,
                )

            def tk_gem_r(nb):
                nc.vector.reduce_sum(
                    out=gem_r2[:, nb : nb + 1],
                    in_=gem[:, 512 * nb : 512 * (nb + 1)],
                    axis=mybir.AxisListType.X,
                )

            def tk_m1(nb):
                lo = 512 * nb
                hi = min(512 * (nb + 1), NT - 1)
                mps = cps.tile([C, 512], F32, tag="cps512", name="mps")
                nc.tensor.matmul(
                    out=mps[:, : hi - lo],
                    lhsT=transT_sb[:],
                    rhs=OH[:, lo + 1 : hi + 1],
                    start=True,
                    stop=True,
                )
                nc.vector.tensor_copy(out=M1[:, lo:hi], in_=mps[:, : hi - lo])

            def tk_m1mult(nb):
                lo = 512 * nb
                hi = min(512 * (nb + 1), NT - 1)
                nc.vector.tensor_tensor(
                    out=M1[:, lo:hi], in0=OH[:, lo:hi], in1=M1[:, lo:hi], op=ALU.mult
                )

            def tk_pd(half):
                nc.vector.reduce_sum(
                    out=pd_r2[:, half : half + 1],
                    in_=pdv[:, 2 * half : 2 * half + 2, 0 : T - 1],
                    axis=mybir.AxisListType.XY,
                )

            def tk_sten():
                st8 = cw.tile([C, GB], F32, tag="st8", name="st8")
                nc.vector.tensor_scalar_mul(out=st8[:], in0=OHv[:, :, 0], scalar1=stv_sb[:])
                nc.vector.reduce_sum(out=st_r[:], in_=st8[:], axis=mybir.AxisListType.X)
                en8 = cw.tile([C, GB], F32, tag="en8", name="en8")
                nc.vector.tensor_scalar_mul(out=en8[:], in0=OHv[:, :, T - 1], scalar1=env_sb[:])
                nc.vector.reduce_sum(out=en_r[:], in_=en8[:], axis=mybir.AxisListType.X)

            def tk_score():
                score_ps = pscr  # reuse the absorber PSUM bank
                parts = (
                    gem_r2[:, 0:1], gem_r2[:, 1:2],
                    pd_r2[:, 0:1], pd_r2[:, 1:2],
                    st_r[:], en_r[:],
                )
                for q, r in enumerate(parts):
                    nc.tensor.matmul(
                        out=score_ps[:1, :1],
                        lhsT=onesC1[:],
                        rhs=r,
                        start=(q == 0),
                        stop=(q == len(parts) - 1),
                        skip_group_check=True,
                    )
                nc.vector.tensor_copy(out=score_sb[:], in_=score_ps[:1, :1])

            tasks = [
                lambda: tk_lab(0), lambda: tk_lab(1),
                lambda: tk_oh(0), lambda: tk_oh(1),
                lambda: tk_gem(0), lambda: tk_gem(1),
                lambda: tk_gem_r(0), lambda: tk_gem_r(1),
                lambda: tk_m1(0), lambda: tk_m1(1),
                lambda: tk_m1mult(0), lambda: tk_m1mult(1),
                lambda: tk_pd(0), lambda: tk_pd(1),
                tk_sten, tk_score,
            ]

            for i in range(TMID):
                if tasks and i % 4 == 1:
                    tasks.pop(0)()
                # --- alpha step t = i+1 (runs for i = 0..TMID-2) ---
                ta = i + 1
                if ta <= TMID - 1:
                    vps = cps.tile([C, GB], F32, tag="vps", bufs=2, name="vps")
                    nc.tensor.matmul(
                        out=vps[:], lhsT=E_sb[:], rhs=v_prev[:], start=True, stop=True
                    )
                    v_new = cw.tile([C, GB], F32, tag="v", name="v_new")
                    qt = qs_a.pop(ta, None)
                    nc.vector.tensor_tensor(
                        out=v_new[:],
                        in0=vps[:],
                        in1=(qt[:] if qt is not None else Qv[:, :, ta]),
                        op=ALU.mult,
                    )
                    v_prev = v_new
                    if ta % RENORM == RENORM - 1 and ta + LAG <= TMID - 1:
                        renorm(v_prev[:], ta + LAG, qs_a, nren_a)
                        nren_a += 1
                # --- beta step t = T-1-i: w = q_t * b_t ; b_{t-1} = E w ---
                tb = T - 1 - i
                w = cw.tile([C, GB], F32, tag="w", name="w")
                qt = qs_b.pop(tb, None)
                qin = qt[:] if qt is not None else Qv[:, :, tb]
                if b_ps is None:
                    nc.vector.tensor_scalar_mul(out=w[:], in0=qin, scalar1=expen_sb[:])
                else:
                    nc.vector.tensor_tensor(out=w[:], in0=qin, in1=b_ps[:], op=ALU.mult)
                b_ps = cps.tile([C, GB], F32, tag="bps", bufs=2, name="b_ps")
                nc.tensor.matmul(out=b_ps[:], lhsT=expTT_sb[:], rhs=w[:], start=True, stop=True)
                j = i + 1  # beta steps completed
                if j % RENORM == 3 and i + 1 + LAG < TMID:
                    renorm(w[:], tb - 1 - LAG, qs_b, 16 + nren_b)
                    nren_b += 1

            # Z = colsum(v_mid * b_mid)
            zv = cw.tile([C, GB], F32, tag="zv")
            nc.vector.tensor_tensor(out=zv[:], in0=v_prev[:], in1=b_ps[:], op=ALU.mult)
            zps = cps.tile([1, GB], F32, tag="cps1", bufs=1)
            nc.tensor.matmul(out=zps[:], lhsT=onesC1[:], rhs=zv[:], start=True, stop=True)
            lnz = cw.tile([1, GB], F32, tag="lnz")
            nc.scalar.activation(out=lnz[:], in_=zps[:], func=AF.Ln)
            lns_all = cw.tile([1, GB, NREN], F32, tag="lns_all")
            nc.scalar.activation(out=lns_all[:], in_=s_store[:], func=AF.Ln)
            off = cw.tile([1, GB], F32, tag="off")
            nc.vector.reduce_sum(out=off[:], in_=lns_all[:], axis=mybir.AxisListType.X)
            logz = cw.tile([1, GB], F32, tag="logz")
            nc.vector.tensor_tensor(out=logz[:], in0=lnz[:], in1=off[:], op=ALU.add)
            lz_tot = cw.tile([1, 1], F32, tag="lztot")
            nc.vector.reduce_sum(out=lz_tot[:], in_=logz[:], axis=mybir.AxisListType.X)
            loss_sb = cw.tile([1, 1], F32, tag="loss_sb")
            nc.vector.tensor_tensor(out=loss_sb[:], in0=lz_tot[:], in1=score_sb[:], op=ALU.subtract)
            nc.sync.dma_start(out=loss_out, in_=loss_sb[:])
            if DEBUG_OUTS:
                dsc = cw.tile([1, 2], F32, tag="dsc")
                nc.vector.tensor_copy(out=dsc[:, 0:1], in_=lz_tot[:])
                nc.vector.tensor_copy(out=dsc[:, 1:2], in_=score_sb[:])
                nc.sync.dma_start(out=dbg["dbg_sc"], in_=dsc[:])
        crf_cm.__exit__(None, None, None)

    est.close()


# ---------------------------------------------------------------------------
# host side
# ---------------------------------------------------------------------------

def make_in_maps(inputs):
    ids = np.asarray(inputs["input_ids"]).astype(np.int64)
    labels = np.asarray(inputs["labels"]).astype(np.int64)
    word_emb = _f32(inputs["word_emb"])
    pos_emb = _f32(inputs["pos_emb"])
    type_emb = _f32(inputs["type_emb"])
    ln_g = _f32(inputs["ln_g"])
    ln_b = _f32(inputs["ln_b"])
    w_ih = _f32(inputs["w_ih"])
    w_hh = _f32(inputs["w_hh"])
    b_ih = _f32(inputs["b_ih"])
    b_hh = _f32(inputs["b_hh"])
    fc_w = _f32(inputs["fc_w"])
    fc_b = _f32(inputs["fc_b"])
    crf_start = _f32(inputs["crf_start"])
    crf_end = _f32(inputs["crf_end"])
    crf_trans = _f32(inputs["crf_trans"])

    posty = np.ascontiguousarray(pos_emb[:T] + type_emb[0][None, :])

    def sig_trick(w, b):
        # fold tanh(x) = 2*sigmoid(2x)-1: scale g-gate rows (2HD:3HD) by 2
        w = w.copy()
        b = b.copy()
        w[2 * HD : 3 * HD] *= 2.0
        b[2 * HD : 3 * HD] *= 2.0
        return w, b

    # per-layer packed weights, both directions: cols [fwd 4HD | bwd 4HD]
    wihT = []
    whhT = []
    biases = []  # [l][d] -> (1536,)
    for l in range(L):
        wl = []
        hl = []
        bl = []
        for d in range(2):
            w = w_ih[l, d]
            bias = b_ih[l, d] + b_hh[l, d]
            if l == 0:
                bias = bias + w @ ln_b
                w = w * ln_g[None, :]
            u = w_hh[l, d]
            w, bias = sig_trick(w, bias)
            u, _ = sig_trick(u, np.zeros(4 * HD, np.float32))
            wl.append(w.T)   # [in_dim, 1536]
            hl.append(u.T)   # [HD, 1536]
            bl.append(bias)
        wihT.append(np.concatenate(wl, axis=1))   # [in_dim, 3072]
        whhT.append(np.concatenate(hl, axis=1))   # [HD, 3072]
        biases.append(bl)

    # b01 [128, 48]: col l*24 + d*12 + m holds bias[l][d][128m:128(m+1)]
    bcols = []
    for l in range(L):
        for d in range(2):
            bcols.append(biases[l][d].reshape(MCH, 128).T)
    b01 = np.ascontiguousarray(np.concatenate(bcols, axis=1))

    cpack = np.zeros((C, 48), np.float32)
    cpack[:, 0:C] = np.exp(crf_trans)
    cpack[:, C : 2 * C] = crf_trans.T
    cpack[:, 28] = np.exp(crf_start)
    cpack[:, 29] = np.exp(crf_end)
    cpack[:, 30] = crf_start
    cpack[:, 31] = crf_end
    cpack[:, 32] = np.arange(C, dtype=np.float32)
    cpack[:, 33] = fc_b
    cpack[:, 34:48] = np.exp(crf_trans).T

    shared = dict(
        word_emb=word_emb,
        posty=posty,
        wih0T=_bf(wihT[0]),
        wih1T=_bf(wihT[1]),
        whh0T=_bf(whhT[0]),
        whh1T=_bf(whhT[1]),
        b01=b01,
        fcT=_bf(fc_w.T),
        cpack=cpack,
    )

    in_maps = []
    for core in range(NCORES):
        sl = slice(GB * core, GB * (core + 1))
        in_maps.append(
            dict(
                ids32=np.ascontiguousarray(ids[sl].reshape(NT, 1).astype(np.int32)),
                labf=np.ascontiguousarray(labels[sl].reshape(1, NT).astype(np.float32)),
                **shared,
            )
        )
    return in_maps


_PROGRAM = None
_COST_MODEL_NS = None


def _get_program():
    global _PROGRAM, _COST_MODEL_NS
    if _PROGRAM is None:
        _PROGRAM = build_program()
        try:
            from concourse.timeline_sim import TimelineSim

            _COST_MODEL_NS = int(TimelineSim(_PROGRAM, trace=False, no_exec=True).simulate())
        except Exception:
            _COST_MODEL_NS = None
    return _PROGRAM


def run(inputs, trace=False):
    nc = _get_program()
    in_maps = make_in_maps(inputs)
    res = run_bass_kernel_spmd(nc, in_maps, core_ids=list(range(NCORES)), trace=trace)
    total = np.float64(0.0)
    for c in range(NCORES):
        total += np.float64(res.results[c]["loss"][0, 0])
    return np.asarray(total, dtype=np.float32), res


def kernel(**inputs):
    out, _ = run(inputs, trace=False)
    return out
